# revision 12
# baseline (speedup 1.0000x reference)
"""Trainium2 Bass kernel for the 3-iteration Snake/deep-snake GNN head.

Strategy: pure data-parallel over the 256 polygon instances (32 per core x 8
cores).  Per core, one fully-unrolled program runs all 3 snake iterations:

  - bilinear gather: cnn_feature is transposed to channel-last on the host;
    on device a single int16-indexed dma_gather per (y-row-set, quarter)
    fetches 8-pixel windows at 4-pixel granularity (global row index
    ind*4096 + y*32 + floor(xb/4) <= 32767 fits int16).
  - the 2x2 bilinear blend is one 5-dim tensor_tensor multiply against
    per-(point,instance) weights + a reduce over (y,slot).
  - circular dilated convs run as 9 shifted matmuls over a halo'd
    [C, inst*160] activation tensor, batched 4 instances per N=512 matmul,
    accumulating in PSUM; BN (eval mode) folds into the ACT relu epilogue.
  - fusion / pred 1x1 convs are plain K-tiled matmuls; the global-max path
    becomes a free-dim reduce_max plus a tiny matmul whose result enters the
    pred1 epilogue as a per-instance ACT bias.
"""

import numpy as np
import ml_dtypes

import concourse.bass as bass
import concourse.bass_isa as bass_isa
import concourse.bacc as bacc
import concourse.mybir as mybir
import concourse.tile as tile
from concourse.bass_utils import run_bass_kernel_spmd
from concourse.masks import make_identity

F32 = mybir.dt.float32
BF16 = mybir.dt.bfloat16
I16 = mybir.dt.int16
I32 = mybir.dt.int32
AL = mybir.AluOpType
AF = mybir.ActivationFunctionType
AX = mybir.AxisListType

B, C, H, W = 8, 64, 128, 128
PTS = 128
RO = 4.0
DIL = [1, 1, 1, 2, 2, 4, 4]
BN_EPS = 1e-5
N_CORES = 8
HALO = PTS + 32  # 16 halo points each side (max dilation 4 * 4 taps)
FEAT_ROWS = B * H * W // 4  # 4-pixel granule rows = 32768

bf16 = ml_dtypes.bfloat16


def _ap(t_ap, extra_off, dims):
    return bass.AP(t_ap.tensor, t_ap.offset + extra_off, dims)


def prep_weights(snake_params):
    """Per snake: transpose conv weights to lhsT layout, fold BN."""
    out = []
    for p in snake_params:
        d = {}
        hw, hb, hg, hbeta, hm, hv = p['head']
        blocks = [(hw, hb, hg, hbeta, hm, hv)] + [tuple(bp) for bp in p['res']]
        tap_ws, s_vecs, bs_vecs, t_vecs = [], [], [], []
        for (w, b, g, beta, m, v) in blocks:
            w = np.asarray(w, np.float32)
            s = np.asarray(g, np.float32) / np.sqrt(np.asarray(v, np.float32) + BN_EPS)
            assert (s > 0).all(), "BN scale must be positive for folded relu"
            tap_ws.append(np.ascontiguousarray(w.transpose(2, 1, 0)).astype(bf16))  # [9, cin, cout]
            s_vecs.append(s)
            bs_vecs.append(np.asarray(b, np.float32) * s)
            t_vecs.append(np.asarray(beta, np.float32) - np.asarray(m, np.float32) * s)
        d['taps'] = tap_ws                      # [0]=head [9,66,128]; 1..7 res [9,128,128]
        d['s'] = np.stack(s_vecs, 1)            # [128, 8]
        d['bs'] = np.stack(bs_vecs, 1)          # [128, 8]
        d['t'] = np.stack(t_vecs, 1)            # [128, 8]
        d['has_t'] = bool(np.any(d['t']))
        fw, fb = p['fusion']
        fw = np.asarray(fw, np.float32)[:, :, 0]           # [256, 1024]
        d['fus_w'] = np.ascontiguousarray(
            fw.T.reshape(8, 128, 256)).astype(bf16)        # [8 ktile, 128, 256]
        d['fus_b'] = np.asarray(fb, np.float32).reshape(2, 128).T.copy()  # [128, 2]
        (w1, b1), (w2, b2), (w3, b3) = p['pred']
        w1 = np.asarray(w1, np.float32)[:, :, 0]           # [256, 1280]
        d['p1_w'] = np.ascontiguousarray(
            w1[:, 256:].T.reshape(8, 128, 256)).astype(bf16)
        d['p1a_w'] = np.ascontiguousarray(
            w1[:, :256].T.reshape(2, 128, 256)).astype(bf16)
        d['p1_b'] = np.asarray(b1, np.float32).reshape(2, 128).T.copy()   # [128, 2]
        w2 = np.asarray(w2, np.float32)[:, :, 0]           # [64, 256]
        d['p2_w'] = np.ascontiguousarray(w2.T.reshape(2, 128, 64)).astype(bf16)
        d['p2_b'] = np.asarray(b2, np.float32).reshape(64, 1).copy()      # [64, 1]
        w3 = np.asarray(w3, np.float32)[:, :, 0]           # [2, 64]
        d['p3_w'] = np.concatenate(
            [w3.T, np.asarray(b3, np.float32)[None, :]], 0).astype(bf16)  # [65, 2]
        out.append(d)
    return out


def build_program(n_inst, n_iter, wmeta):
    """Build the per-core Bass program.  n_inst instances, n_iter snake iters."""
    assert n_inst % 4 == 0
    NQ = max(1, n_inst // 8)         # gather quarters
    QI = n_inst // NQ                # instances per quarter (8)
    NH = max(1, n_inst // 16)        # psum halves
    HI = n_inst // NH                # instances per half (16)
    NCH = HI // 4                    # 4-instance chunks per half

    nc = bacc.Bacc("TRN2", target_bir_lowering=False, debug=False,
                   num_devices=N_CORES, name="snake")
    feat = nc.dram_tensor("feat", [FEAT_ROWS * 256 + 512], BF16, kind="ExternalInput")
    polys_pt = nc.dram_tensor("polys_pt", [PTS, n_inst * 2], F32, kind="ExternalInput")
    ind_rep = nc.dram_tensor("ind_rep", [PTS, n_inst], F32, kind="ExternalInput")
    out_pys = nc.dram_tensor("out_pys", [n_iter, PTS, n_inst * 2], F32,
                             kind="ExternalOutput")
    idx_scr = nc.dram_tensor("idx_scr", [2 * n_inst * PTS], I16, kind="Internal")

    wd = []  # per-snake dram tensors
    for k in range(n_iter):
        m = wmeta[k]
        e = {}
        e['head'] = nc.dram_tensor(f"w{k}_head", [9, 66, 128], BF16, kind="ExternalInput")
        for i in range(7):
            e[f'res{i}'] = nc.dram_tensor(f"w{k}_res{i}", [9, 128, 128], BF16, kind="ExternalInput")
        e['s'] = nc.dram_tensor(f"w{k}_s", [128, 8], F32, kind="ExternalInput")
        e['bs'] = nc.dram_tensor(f"w{k}_bs", [128, 8], F32, kind="ExternalInput")
        e['t'] = nc.dram_tensor(f"w{k}_t", [128, 8], F32, kind="ExternalInput")
        e['fus_w'] = nc.dram_tensor(f"w{k}_fus", [8, 128, 256], BF16, kind="ExternalInput")
        e['fus_b'] = nc.dram_tensor(f"w{k}_fusb", [128, 2], F32, kind="ExternalInput")
        e['p1_w'] = nc.dram_tensor(f"w{k}_p1", [8, 128, 256], BF16, kind="ExternalInput")
        e['p1a_w'] = nc.dram_tensor(f"w{k}_p1a", [2, 128, 256], BF16, kind="ExternalInput")
        e['p1_b'] = nc.dram_tensor(f"w{k}_p1b", [128, 2], F32, kind="ExternalInput")
        e['p2_w'] = nc.dram_tensor(f"w{k}_p2", [2, 128, 64], BF16, kind="ExternalInput")
        e['p2_b'] = nc.dram_tensor(f"w{k}_p2b", [64, 1], F32, kind="ExternalInput")
        e['p3_w'] = nc.dram_tensor(f"w{k}_p3", [65, 2], BF16, kind="ExternalInput")
        wd.append(e)

    with tile.TileContext(nc) as tc:
        with (
            tc.tile_pool(name="xpool", bufs=1) as xpool,
            tc.tile_pool(name="coord", bufs=1) as cpool,
            tc.tile_pool(name="gpool", bufs=1) as gpool,
            tc.tile_pool(name="wstream", bufs=2) as wpool,
            tc.tile_pool(name="psum", bufs=8, space="PSUM") as psum,
        ):
            ident = cpool.tile([128, 128], F32)
            make_identity(nc, ident[:])

            # persistent activation tensors
            X = [xpool.tile([66, n_inst, HALO], BF16, tag="X0", name="X0")] + [
                xpool.tile([128, n_inst, HALO], BF16, tag=f"X{i}", name=f"X{i}") for i in range(1, 9)]
            P1 = [xpool.tile([128, n_inst, 128], BF16, tag=f"P1_{mo}", name=f"P1_{mo}") for mo in range(2)]
            P2 = xpool.tile([65, n_inst, 128], BF16)
            nc.vector.memset(P2[64:65, :, :], 1.0)
            blend = xpool.tile([128, n_inst, 66], F32)

            # coord tiles
            py = cpool.tile([128, n_inst, 2], F32)
            py_img = cpool.tile([128, n_inst, 2], F32)
            can = cpool.tile([128, n_inst, 2], F32)
            pyneg = cpool.tile([128, n_inst, 2], F32)
            mx = cpool.tile([128, n_inst, 2], F32)
            idx_basef = cpool.tile([128, n_inst], F32)
            W10q = [cpool.tile([128, 2, n_inst // max(1, n_inst // 8), 5], F32,
                               tag=f"W10q{h}", name=f"W10q{h}")
                    for h in range(max(1, n_inst // 8))]
            idx_f = cpool.tile([128, n_inst * 2], F32)
            idx_i = cpool.tile([128, n_inst * 2], I16)
            idxw = cpool.tile([128, n_inst * 16], I16)
            gmax = [cpool.tile([128, n_inst], F32, tag=f"gmax{mo}", name=f"gmax{mo}") for mo in range(2)]
            gmax_bf = [cpool.tile([128, n_inst], BF16, tag=f"gmaxb{mo}", name=f"gmaxb{mo}") for mo in range(2)]
            vb1 = [cpool.tile([128, n_inst], F32, tag=f"vb1{mo}", name=f"vb1{mo}") for mo in range(2)]

            # scratch for coord math
            def ctile(tag):
                return cpool.tile([128, n_inst], F32, tag=tag, name=tag)

            # per-snake bias/scale tiles (persistent, tiny)
            svec, bsvec, tvec, fusb, p1b, p2b = [], [], [], [], [], []
            for k in range(n_iter):
                s_t = cpool.tile([128, 8], F32, tag=f"s{k}")
                bs_t = cpool.tile([128, 8], F32, tag=f"bs{k}")
                t_t = cpool.tile([128, 8], F32, tag=f"t{k}")
                fb_t = cpool.tile([128, 2], F32, tag=f"fb{k}")
                p1b_t = cpool.tile([128, 2], F32, tag=f"p1b{k}")
                p2b_t = cpool.tile([64, 1], F32, tag=f"p2b{k}")
                nc.sync.dma_start(out=s_t[:], in_=wd[k]['s'][:])
                nc.sync.dma_start(out=bs_t[:], in_=wd[k]['bs'][:])
                nc.sync.dma_start(out=t_t[:], in_=wd[k]['t'][:])
                nc.sync.dma_start(out=fb_t[:], in_=wd[k]['fus_b'][:])
                nc.sync.dma_start(out=p1b_t[:], in_=wd[k]['p1_b'][:])
                nc.sync.dma_start(out=p2b_t[:], in_=wd[k]['p2_b'][:])
                svec.append(s_t); bsvec.append(bs_t); tvec.append(t_t)
                fusb.append(fb_t); p1b.append(p1b_t); p2b.append(p2b_t)

            # init py = clip(polys/4, 0, 127); idx_base = ind*4096
            nc.sync.dma_start(out=py[:], in_=polys_pt[:].rearrange("p (n c) -> p n c", c=2))
            nc.vector.tensor_scalar(py[:], py[:], 0.25, None, AL.mult)
            nc.vector.tensor_scalar(py[:], py[:], 127.0, 0.0, AL.min, AL.max)
            nc.sync.dma_start(out=idx_basef[:], in_=ind_rep[:])
            nc.vector.tensor_scalar(idx_basef[:], idx_basef[:], 4096.0, None, AL.mult)

            feat_win = _ap(feat[:], 0, [[256, FEAT_ROWS], [1, 512]])

            for k in range(n_iter):
                wk = wd[k]
                # ---------------- A: coordinate / weight / index math (PT layout)
                px = _ap(py[:], 0, [[n_inst * 2, 128], [2, n_inst]])
                pyy = _ap(py[:], 1, [[n_inst * 2, 128], [2, n_inst]])

                t_x = ctile("t_x"); x0i = cpool.tile([128, n_inst], I32, tag="x0i", name="x0i")
                x0f = ctile("x0f"); wx1 = ctile("wx1"); xb = ctile("xb")
                vx0 = ctile("vx0"); vx1 = ctile("vx1"); tmp = ctile("tmp")
                aw = ctile("aw"); bw = ctile("bw")
                e_eq = ctile("e_eq"); e_lo = ctile("e_lo"); e_hi = ctile("e_hi")
                ws0 = ctile("ws0"); ws1 = ctile("ws1")
                xq = ctile("xq"); xqi = cpool.tile([128, n_inst], I32, tag="xqi", name="xqi")
                s_off = ctile("s_off")
                m_prev = None

                nc.vector.tensor_scalar(t_x[:], px, 1.0, None, AL.subtract)
                nc.vector.tensor_copy(x0i[:], t_x[:])
                nc.vector.tensor_copy(x0f[:], x0i[:])
                nc.vector.tensor_scalar(t_x[:], px, 0.5, None, AL.subtract)
                nc.vector.tensor_tensor(out=wx1[:], in0=t_x[:], in1=x0f[:], op=AL.subtract)
                nc.vector.tensor_scalar(xb[:], x0f[:], 126.0, 0.0, AL.min, AL.max)
                nc.vector.tensor_scalar(vx0[:], x0f[:], 0.0, None, AL.is_ge)
                nc.vector.tensor_scalar(tmp[:], x0f[:], 127.0, None, AL.is_le)
                nc.vector.tensor_tensor(out=vx0[:], in0=vx0[:], in1=tmp[:], op=AL.mult)
                nc.vector.tensor_scalar(vx1[:], x0f[:], -1.0, None, AL.is_ge)
                nc.vector.tensor_scalar(tmp[:], x0f[:], 126.0, None, AL.is_le)
                nc.vector.tensor_tensor(out=vx1[:], in0=vx1[:], in1=tmp[:], op=AL.mult)
                nc.vector.tensor_scalar(aw[:], wx1[:], -1.0, 1.0, AL.mult, AL.add)
                nc.vector.tensor_tensor(out=aw[:], in0=aw[:], in1=vx0[:], op=AL.mult)
                nc.vector.tensor_tensor(out=bw[:], in0=wx1[:], in1=vx1[:], op=AL.mult)
                nc.vector.tensor_tensor(out=e_eq[:], in0=x0f[:], in1=xb[:], op=AL.is_equal)
                nc.vector.tensor_tensor(out=e_lo[:], in0=x0f[:], in1=xb[:], op=AL.is_lt)
                nc.vector.tensor_tensor(out=e_hi[:], in0=x0f[:], in1=xb[:], op=AL.is_gt)
                nc.vector.tensor_tensor(out=ws0[:], in0=aw[:], in1=e_eq[:], op=AL.mult)
                nc.vector.tensor_tensor(out=tmp[:], in0=bw[:], in1=e_lo[:], op=AL.mult)
                nc.vector.tensor_tensor(out=ws0[:], in0=ws0[:], in1=tmp[:], op=AL.add)
                nc.vector.tensor_tensor(out=ws1[:], in0=bw[:], in1=e_eq[:], op=AL.mult)
                nc.vector.tensor_tensor(out=tmp[:], in0=aw[:], in1=e_hi[:], op=AL.mult)
                nc.vector.tensor_tensor(out=ws1[:], in0=ws1[:], in1=tmp[:], op=AL.add)
                # xq = floor(xb/4) via round((xb-1.5)*0.25); s_off = xb - 4*xq
                nc.vector.tensor_scalar(xq[:], xb[:], 1.5, 0.25, AL.subtract, AL.mult)
                nc.vector.tensor_copy(xqi[:], xq[:])
                nc.vector.tensor_copy(xq[:], xqi[:])
                nc.vector.tensor_scalar(s_off[:], xq[:], -4.0, None, AL.mult)
                nc.vector.tensor_tensor(out=s_off[:], in0=s_off[:], in1=xb[:], op=AL.add)

                # y side
                y0f = ctile("y0f"); wy1 = ctile("wy1")
                yb0 = ctile("yb0"); yb1 = ctile("yb1")
                vy0 = ctile("vy0"); vy1 = ctile("vy1")
                wy0v = ctile("wy0v"); wy1v = ctile("wy1v")
                nc.vector.tensor_scalar(t_x[:], pyy, 1.0, None, AL.subtract)
                nc.vector.tensor_copy(x0i[:], t_x[:])
                nc.vector.tensor_copy(y0f[:], x0i[:])
                nc.vector.tensor_scalar(t_x[:], pyy, 0.5, None, AL.subtract)
                nc.vector.tensor_tensor(out=wy1[:], in0=t_x[:], in1=y0f[:], op=AL.subtract)
                nc.vector.tensor_scalar(yb0[:], y0f[:], 127.0, 0.0, AL.min, AL.max)
                nc.vector.tensor_scalar(yb1[:], y0f[:], 1.0, None, AL.add)
                nc.vector.tensor_scalar(yb1[:], yb1[:], 127.0, 0.0, AL.min, AL.max)
                nc.vector.tensor_scalar(vy0[:], y0f[:], 0.0, None, AL.is_ge)
                nc.vector.tensor_scalar(tmp[:], y0f[:], 127.0, None, AL.is_le)
                nc.vector.tensor_tensor(out=vy0[:], in0=vy0[:], in1=tmp[:], op=AL.mult)
                nc.vector.tensor_scalar(vy1[:], y0f[:], -1.0, None, AL.is_ge)
                nc.vector.tensor_scalar(tmp[:], y0f[:], 126.0, None, AL.is_le)
                nc.vector.tensor_tensor(out=vy1[:], in0=vy1[:], in1=tmp[:], op=AL.mult)
                nc.vector.tensor_scalar(wy0v[:], wy1[:], -1.0, 1.0, AL.mult, AL.add)
                nc.vector.tensor_tensor(out=wy0v[:], in0=wy0v[:], in1=vy0[:], op=AL.mult)
                nc.vector.tensor_tensor(out=wy1v[:], in0=wy1[:], in1=vy1[:], op=AL.mult)

                # W10h[h][p, y, n, s] = wy_yv * (ws0*[s_off==s] + ws1*[s_off==s-1])
                for s in range(5):
                    first = True
                    if s < 4:
                        nc.vector.tensor_scalar(tmp[:], s_off[:], float(s), None, AL.is_equal)
                        nc.vector.tensor_tensor(out=e_eq[:], in0=ws0[:], in1=tmp[:], op=AL.mult)
                        first = False
                    if s >= 1:
                        nc.vector.tensor_scalar(tmp[:], s_off[:], float(s - 1), None, AL.is_equal)
                        nc.vector.tensor_tensor(out=tmp[:], in0=ws1[:], in1=tmp[:], op=AL.mult)
                        if first:
                            nc.vector.tensor_copy(e_eq[:], tmp[:])
                        else:
                            nc.vector.tensor_tensor(out=e_eq[:], in0=e_eq[:], in1=tmp[:], op=AL.add)
                    # e_eq now holds the x-part slot weight for slot s
                    for q in range(NQ):
                        qp = 2 * QI * 5
                        for yi, wyv in enumerate([wy0v, wy1v]):
                            dst = _ap(W10q[q][:], yi * QI * 5 + s, [[qp, 128], [5, QI]])
                            nc.vector.tensor_tensor(out=dst, in0=e_eq[:, q * QI:(q + 1) * QI],
                                                    in1=wyv[:, q * QI:(q + 1) * QI], op=AL.mult)

                # gather indices: idx_y = ind*4096 + yb*32 + xq
                for yi, yb in enumerate([yb0, yb1]):
                    sl = idx_f[:, yi * n_inst:(yi + 1) * n_inst]
                    nc.vector.tensor_scalar(sl, yb[:], 32.0, None, AL.mult)
                    nc.vector.tensor_tensor(out=sl, in0=sl, in1=idx_basef[:], op=AL.add)
                    nc.vector.tensor_tensor(out=sl, in0=sl, in1=xq[:], op=AL.add)
                nc.vector.tensor_copy(idx_i[:], idx_f[:])

                # ---------------- B: index rearrangement roundtrip through DRAM
                nc.sync.dma_start(
                    out=idx_scr[:2 * n_inst * 128].rearrange("(s p) -> p s", p=128),
                    in_=idx_i[:])
                for r in range(8):
                    nc.sync.dma_start(
                        out=idxw[16 * r:16 * (r + 1), :],
                        in_=idx_scr[:2 * n_inst * 128].rearrange("(c r) -> r c", r=16))

                # ---------------- C+D: can (needs gpsimd all-reduce) for blend cols 64:65
                nc.vector.tensor_scalar(pyneg[:], py[:], -1.0, None, AL.mult)
                nc.gpsimd.partition_all_reduce(mx[:], pyneg[:], 128,
                                               bass_isa.ReduceOp.max)
                nc.vector.tensor_tensor(out=can[:], in0=py[:], in1=mx[:], op=AL.add)
                bl_can = _ap(blend[:], 64, [[n_inst * 66, 128], [66, n_inst], [1, 2]])
                nc.vector.tensor_scalar(bl_can, can[:], 4.0, None, AL.mult)

                # ---------------- E: gather + blend per quarter
                for q in range(NQ):
                    g_t = gpool.tile([128, 2, QI, 512], BF16, tag="gq", name="g_t")
                    for yi in range(2):
                        c0 = yi * n_inst * 8 + q * QI * 8
                        nc.gpsimd.dma_gather(
                            g_t[:, yi, :, :], feat_win, idxw[:, c0:c0 + QI * 8],
                            QI * 128, QI * 128, 512, elem_step=256)
                    prod = gpool.tile([128, 2 * QI, 64, 5], BF16, tag="prod", name="prod")
                    gp = 2 * QI * 512
                    # logical order (y-inst merged, ch, slot)
                    in0 = _ap(g_t[:], 0, [[gp, 128], [512, 2 * QI], [1, 64], [64, 5]])
                    in1 = _ap(W10q[q][:], 0, [[2 * QI * 5, 128], [5, 2 * QI], [0, 64], [1, 5]])
                    nc.vector.tensor_tensor(out=prod[:], in0=in0, in1=in1, op=AL.mult)
                    r1 = gpool.tile([128, 2 * QI, 64], F32, tag="r1", name="r1")
                    nc.vector.tensor_reduce(out=r1[:], in_=prod[:], axis=AX.X, op=AL.add)
                    # second reduce over y: view r1 as (inst, ch, y)
                    r1v = _ap(r1[:], 0, [[2 * QI * 64, 128], [64, QI], [1, 64], [QI * 64, 2]])
                    red = _ap(blend[:], q * QI * 66,
                              [[n_inst * 66, 128], [66, QI], [1, 64]])
                    nc.vector.tensor_reduce(out=red, in_=r1v, axis=AX.X, op=AL.add)

                # ---------------- F: transpose blend -> X0 (+ halo)
                for n in range(n_inst):
                    ps_t = psum.tile([66, 128], F32, tag="ps", space="PSUM")
                    nc.tensor.transpose(out=ps_t[:], in_=blend[:, n, :], identity=ident[:])
                    nc.scalar.copy(X[0][:, n, 16:144], ps_t[:])
                nc.vector.tensor_copy(X[0][:, :, 0:16], X[0][:, :, 128:144])
                nc.vector.tensor_copy(X[0][:, :, 144:160], X[0][:, :, 16:32])

                # ---------------- G: conv blocks
                for blk in range(8):
                    Xin, Xout = X[blk], X[blk + 1]
                    K = 66 if blk == 0 else 128
                    dil = 1 if blk == 0 else DIL[blk - 1]
                    wsrc = wk['head'] if blk == 0 else wk[f'res{blk - 1}']
                    taps = wpool.tile([K, 9, 128], BF16, tag="taps")
                    nc.sync.dma_start(out=taps[:], in_=wsrc[:, :K, :].rearrange("t k m -> k t m"))
                    s_ap = svec[k][:, blk:blk + 1]
                    bs_ap = bsvec[k][:, blk:blk + 1]
                    for h in range(NH):
                        pst = [psum.tile([128, 4 * 128], F32, tag="ps", space="PSUM", name=f"pst{h}_{i}")
                               for i in range(NCH)]
                        for t in range(9):
                            sh = 16 + (t - 4) * dil
                            for cch in range(NCH):
                                g0 = h * HI + cch * 4
                                rhs = Xin[:K, g0:g0 + 4, sh:sh + 128]
                                nc.tensor.matmul(pst[cch][:], taps[:, t, :], rhs,
                                                 start=(t == 0), stop=(t == 8))
                        for cch in range(NCH):
                            g0 = h * HI + cch * 4
                            octr = Xout[:, g0:g0 + 4, 16:144]
                            nc.scalar.activation(octr, pst[cch][:].rearrange("p (g n) -> p g n", g=4),
                                                 AF.Relu, bias=bs_ap, scale=s_ap)
                            if wmeta[k]['has_t']:
                                nc.vector.tensor_scalar(octr, octr, tvec[k][:, blk:blk + 1],
                                                        None, AL.add)
                            if blk > 0:
                                nc.vector.tensor_tensor(out=octr, in0=octr,
                                                        in1=Xin[:, g0:g0 + 4, 16:144], op=AL.add)
                    nc.vector.tensor_copy(Xout[:, :, 0:16], Xout[:, :, 128:144])
                    nc.vector.tensor_copy(Xout[:, :, 144:160], Xout[:, :, 16:32])

                # ---------------- H: fusion conv + global max
                fusw = wpool.tile([128, 8, 256], BF16, tag="fus")
                nc.sync.dma_start(out=fusw[:], in_=wk['fus_w'][:].rearrange("s k m -> k s m"))
                for h in range(NH):
                    for cch in range(NCH):
                        g0 = h * HI + cch * 4
                        for mo in range(2):
                            pf = psum.tile([128, 4 * 128], F32, tag="ps", space="PSUM")
                            for s in range(8):
                                rhs = X[s + 1][:, g0:g0 + 4, 16:144]
                                nc.tensor.matmul(pf[:], fusw[:, s, mo * 128:(mo + 1) * 128],
                                                 rhs, start=(s == 0), stop=(s == 7))
                            nc.vector.tensor_reduce(
                                out=gmax[mo][:, g0:g0 + 4],
                                in_=pf[:].rearrange("p (g n) -> p g n", g=4),
                                axis=AX.X, op=AL.max)
                for mo in range(2):
                    nc.vector.tensor_scalar(gmax[mo][:], gmax[mo][:],
                                            fusb[k][:, mo:mo + 1], None, AL.add)
                    nc.vector.tensor_copy(gmax_bf[mo][:], gmax[mo][:])

                # v = W1a @ g + b1  (per-instance pred1 bias)
                p1aw = wpool.tile([128, 2, 256], BF16, tag="p1a")
                nc.sync.dma_start(out=p1aw[:], in_=wk['p1a_w'][:].rearrange("s k m -> k s m"))
                for mo in range(2):
                    vps = psum.tile([128, n_inst], F32, tag="ps", space="PSUM")
                    for ki in range(2):
                        nc.tensor.matmul(vps[:], p1aw[:, ki, mo * 128:(mo + 1) * 128],
                                         gmax_bf[ki][:], start=(ki == 0), stop=(ki == 1))
                    nc.vector.tensor_scalar(vb1[mo][:], vps[:], p1b[k][:, mo:mo + 1],
                                            None, AL.add)

                # ---------------- I: pred1
                p1w = wpool.tile([128, 8, 256], BF16, tag="p1")
                nc.sync.dma_start(out=p1w[:], in_=wk['p1_w'][:].rearrange("s k m -> k s m"))
                for h in range(NH):
                    for cch in range(NCH):
                        g0 = h * HI + cch * 4
                        for mo in range(2):
                            pp = psum.tile([128, 4 * 128], F32, tag="ps", space="PSUM")
                            for s in range(8):
                                rhs = X[s + 1][:, g0:g0 + 4, 16:144]
                                nc.tensor.matmul(pp[:], p1w[:, s, mo * 128:(mo + 1) * 128],
                                                 rhs, start=(s == 0), stop=(s == 7))
                            for j in range(4):
                                n = g0 + j
                                nc.scalar.activation(
                                    P1[mo][:, n, :], pp[:, j * 128:(j + 1) * 128],
                                    AF.Relu, bias=vb1[mo][:, n:n + 1], scale=1.0)

                # ---------------- J: pred2
                p2w = wpool.tile([128, 2, 64], BF16, tag="p2")
                nc.sync.dma_start(out=p2w[:], in_=wk['p2_w'][:].rearrange("s k m -> k s m"))
                for h in range(NH):
                    for cch in range(NCH):
                        g0 = h * HI + cch * 4
                        pp = psum.tile([64, 4 * 128], F32, tag="ps", space="PSUM")
                        for ki in range(2):
                            nc.tensor.matmul(pp[:], p2w[:, ki, :], P1[ki][:, g0:g0 + 4, :],
                                             start=(ki == 0), stop=(ki == 1))
                        nc.scalar.activation(P2[:64, g0:g0 + 4, :],
                                             pp[:].rearrange("p (g n) -> p g n", g=4),
                                             AF.Relu, bias=p2b[k][:, 0:1], scale=1.0)

                # ---------------- K: pred3 (per-instance lhsT) + py update
                p3w = wpool.tile([65, 2], BF16, tag="p3")
                nc.sync.dma_start(out=p3w[:], in_=wk['p3_w'][:])
                p3ps = psum.tile([128, n_inst * 2], F32, tag="ps", space="PSUM")
                for n in range(n_inst):
                    nc.tensor.matmul(p3ps[:, 2 * n:2 * n + 2], P2[:, n, :], p3w[:],
                                     start=True, stop=True, skip_group_check=True)
                nc.vector.tensor_scalar(py_img[:], py[:], 4.0, None, AL.mult)
                nc.vector.tensor_tensor(
                    out=py_img[:], in0=py_img[:],
                    in1=p3ps[:].rearrange("p (n c) -> p n c", c=2), op=AL.add)
                nc.sync.dma_start(
                    out=out_pys[k].rearrange("p (n c) -> p n c", c=2), in_=py_img[:])
                if k + 1 < n_iter:
                    nc.vector.tensor_scalar(py[:], py_img[:], 0.25, None, AL.mult)

    nc.compile()
    return nc


_PROGRAM_CACHE = {}


def kernel(cnn_feature, polys, ind, snake_params):
    cnn_feature = np.asarray(cnn_feature, np.float32)
    polys = np.asarray(polys, np.float32)
    ind = np.asarray(ind)
    n_total, n_pts = polys.shape[0], polys.shape[1]
    assert n_pts == PTS and cnn_feature.shape == (B, C, H, W)
    n_inst = n_total // N_CORES
    n_iter = len(snake_params)

    wmeta = prep_weights(snake_params)

    key = (n_inst, n_iter)
    if key not in _PROGRAM_CACHE:
        _PROGRAM_CACHE[key] = build_program(n_inst, n_iter, wmeta)
    nc = _PROGRAM_CACHE[key]

    # host data prep
    feat_t = np.ascontiguousarray(cnn_feature.transpose(0, 2, 3, 1)).reshape(-1, 64)
    feat_pad = np.zeros((FEAT_ROWS * 256 + 512,), bf16)
    feat_pad[:feat_t.size] = feat_t.reshape(-1).astype(bf16)

    wmap = {}
    for k in range(n_iter):
        m = wmeta[k]
        wmap[f"w{k}_head"] = np.zeros((9, 66, 128), bf16)
        wmap[f"w{k}_head"][:, :66, :] = m['taps'][0]
        for i in range(7):
            wmap[f"w{k}_res{i}"] = m['taps'][i + 1]
        wmap[f"w{k}_s"] = m['s']; wmap[f"w{k}_bs"] = m['bs']; wmap[f"w{k}_t"] = m['t']
        wmap[f"w{k}_fus"] = m['fus_w']; wmap[f"w{k}_fusb"] = m['fus_b']
        wmap[f"w{k}_p1"] = m['p1_w']; wmap[f"w{k}_p1a"] = m['p1a_w']
        wmap[f"w{k}_p1b"] = m['p1_b']
        wmap[f"w{k}_p2"] = m['p2_w']; wmap[f"w{k}_p2b"] = m['p2_b']
        wmap[f"w{k}_p3"] = m['p3_w']

    in_maps = []
    for c in range(N_CORES):
        sl = slice(c * n_inst, (c + 1) * n_inst)
        pp = polys[sl]  # [n_inst, 128, 2]
        polys_pt = np.ascontiguousarray(pp.transpose(1, 0, 2)).reshape(PTS, n_inst * 2)
        ind_rep = np.broadcast_to(ind[sl].astype(np.float32)[None, :],
                                  (PTS, n_inst)).copy()
        m = {"feat": feat_pad, "polys_pt": polys_pt, "ind_rep": ind_rep}
        m.update(wmap)
        in_maps.append(m)

    res = run_bass_kernel_spmd(nc, in_maps, core_ids=list(range(N_CORES)))
    outs = []
    for c in range(N_CORES):
        o = res.results[c]["out_pys"]  # [n_iter, 128, n_inst*2]
        outs.append(o.reshape(n_iter, PTS, n_inst, 2).transpose(0, 2, 1, 3))
    return np.concatenate(outs, axis=1).astype(np.float32)  # [n_iter, N, P, 2]


# revision 14
# speedup vs baseline: 86.2915x; 86.2915x over previous
"""Trainium2 Bass kernel for the 3-iteration Snake/deep-snake GNN head.

Strategy: pure data-parallel over the 256 polygon instances (32 per core x 8
cores).  Per core, one fully-unrolled program runs all 3 snake iterations:

  - bilinear gather: cnn_feature is transposed to channel-last on the host;
    on device a single int16-indexed dma_gather per (y-row-set, quarter)
    fetches 8-pixel windows at 4-pixel granularity (global row index
    ind*4096 + y*32 + floor(xb/4) <= 32767 fits int16).
  - the 2x2 bilinear blend is one 5-dim tensor_tensor multiply against
    per-(point,instance) weights + a reduce over (y,slot).
  - circular dilated convs run as 9 shifted matmuls over a halo'd
    [C, inst*160] activation tensor, batched 4 instances per N=512 matmul,
    accumulating in PSUM; BN (eval mode) folds into the ACT relu epilogue.
  - fusion / pred 1x1 convs are plain K-tiled matmuls; the global-max path
    becomes a free-dim reduce_max plus a tiny matmul whose result enters the
    pred1 epilogue as a per-instance ACT bias.
"""

import numpy as np
import ml_dtypes

import concourse.bass as bass
import concourse.bass_isa as bass_isa
import concourse.bacc as bacc
import concourse.mybir as mybir
import concourse.tile as tile
from concourse.bass_utils import run_bass_kernel_spmd
from concourse.masks import make_identity

F32 = mybir.dt.float32
BF16 = mybir.dt.bfloat16
I16 = mybir.dt.int16
I32 = mybir.dt.int32
AL = mybir.AluOpType
AF = mybir.ActivationFunctionType
AX = mybir.AxisListType

B, C, H, W = 8, 64, 128, 128
PTS = 128
RO = 4.0
DIL = [1, 1, 1, 2, 2, 4, 4]
BN_EPS = 1e-5
N_CORES = 8
HALO = PTS + 32  # 16 halo points each side (max dilation 4 * 4 taps)
FEAT_ROWS = B * H * W // 4  # 4-pixel granule rows = 32768

bf16 = ml_dtypes.bfloat16


def _ap(t_ap, extra_off, dims):
    return bass.AP(t_ap.tensor, t_ap.offset + extra_off, dims)


def prep_weights(snake_params):
    """Per snake: transpose conv weights to lhsT layout, fold BN."""
    out = []
    for p in snake_params:
        d = {}
        hw, hb, hg, hbeta, hm, hv = p['head']
        blocks = [(hw, hb, hg, hbeta, hm, hv)] + [tuple(bp) for bp in p['res']]
        tap_ws, s_vecs, bs_vecs, t_vecs = [], [], [], []
        for (w, b, g, beta, m, v) in blocks:
            w = np.asarray(w, np.float32)
            s = np.asarray(g, np.float32) / np.sqrt(np.asarray(v, np.float32) + BN_EPS)
            assert (s > 0).all(), "BN scale must be positive for folded relu"
            tap_ws.append(np.ascontiguousarray(w.transpose(2, 1, 0)).astype(bf16))  # [9, cin, cout]
            s_vecs.append(s)
            bs_vecs.append(np.asarray(b, np.float32) * s)
            t_vecs.append(np.asarray(beta, np.float32) - np.asarray(m, np.float32) * s)
        d['taps'] = tap_ws                      # [0]=head [9,66,128]; 1..7 res [9,128,128]
        d['s'] = np.stack(s_vecs, 1)            # [128, 8]
        d['bs'] = np.stack(bs_vecs, 1)          # [128, 8]
        d['t'] = np.stack(t_vecs, 1)            # [128, 8]
        d['has_t'] = bool(np.any(d['t']))
        fw, fb = p['fusion']
        fw = np.asarray(fw, np.float32)[:, :, 0]           # [256, 1024]
        d['fus_w'] = np.ascontiguousarray(
            fw.T.reshape(8, 128, 256)).astype(bf16)        # [8 ktile, 128, 256]
        d['fus_b'] = np.asarray(fb, np.float32).reshape(2, 128).T.copy()  # [128, 2]
        (w1, b1), (w2, b2), (w3, b3) = p['pred']
        w1 = np.asarray(w1, np.float32)[:, :, 0]           # [256, 1280]
        d['p1_w'] = np.ascontiguousarray(
            w1[:, 256:].T.reshape(8, 128, 256)).astype(bf16)
        d['p1a_w'] = np.ascontiguousarray(
            w1[:, :256].T.reshape(2, 128, 256)).astype(bf16)
        d['p1_b'] = np.asarray(b1, np.float32).reshape(2, 128).T.copy()   # [128, 2]
        w2 = np.asarray(w2, np.float32)[:, :, 0]           # [64, 256]
        d['p2_w'] = np.ascontiguousarray(w2.T.reshape(2, 128, 64)).astype(bf16)
        d['p2_b'] = np.asarray(b2, np.float32).reshape(64, 1).copy()      # [64, 1]
        w3 = np.asarray(w3, np.float32)[:, :, 0]           # [2, 64]
        d['p3_w'] = np.concatenate(
            [w3.T, np.asarray(b3, np.float32)[None, :]], 0).astype(bf16)  # [65, 2]
        out.append(d)
    return out


def build_program(n_inst, n_iter, wmeta):
    """Build the per-core Bass program.  n_inst instances, n_iter snake iters."""
    assert n_inst % 4 == 0
    NQ = max(1, n_inst // 4)         # gather groups
    QI = n_inst // NQ                # instances per gather group (4)
    NH = max(1, n_inst // 16)        # psum halves
    HI = n_inst // NH                # instances per half (16)
    NCH = HI // 4                    # 4-instance chunks per half

    nc = bacc.Bacc("TRN2", target_bir_lowering=False, debug=False,
                   num_devices=N_CORES, name="snake")
    feat = nc.dram_tensor("feat", [FEAT_ROWS * 256 + 512], BF16, kind="ExternalInput")
    polys_pt = nc.dram_tensor("polys_pt", [PTS, n_inst * 2], F32, kind="ExternalInput")
    ind_rep = nc.dram_tensor("ind_rep", [PTS, n_inst], F32, kind="ExternalInput")
    out_pys = nc.dram_tensor("out_pys", [n_iter, PTS, n_inst * 2], F32,
                             kind="ExternalOutput")
    idx_scr = nc.dram_tensor("idx_scr", [2 * n_inst * PTS], I16, kind="Internal")

    wd = []  # per-snake dram tensors
    for k in range(n_iter):
        m = wmeta[k]
        e = {}
        e['head'] = nc.dram_tensor(f"w{k}_head", [9, 66, 128], BF16, kind="ExternalInput")
        for i in range(7):
            e[f'res{i}'] = nc.dram_tensor(f"w{k}_res{i}", [9, 128, 128], BF16, kind="ExternalInput")
        e['s'] = nc.dram_tensor(f"w{k}_s", [128, 8], F32, kind="ExternalInput")
        e['bs'] = nc.dram_tensor(f"w{k}_bs", [128, 8], F32, kind="ExternalInput")
        e['t'] = nc.dram_tensor(f"w{k}_t", [128, 8], F32, kind="ExternalInput")
        e['fus_w'] = nc.dram_tensor(f"w{k}_fus", [8, 128, 256], BF16, kind="ExternalInput")
        e['fus_b'] = nc.dram_tensor(f"w{k}_fusb", [128, 2], F32, kind="ExternalInput")
        e['p1_w'] = nc.dram_tensor(f"w{k}_p1", [8, 128, 256], BF16, kind="ExternalInput")
        e['p1a_w'] = nc.dram_tensor(f"w{k}_p1a", [2, 128, 256], BF16, kind="ExternalInput")
        e['p1_b'] = nc.dram_tensor(f"w{k}_p1b", [128, 2], F32, kind="ExternalInput")
        e['p2_w'] = nc.dram_tensor(f"w{k}_p2", [2, 128, 64], BF16, kind="ExternalInput")
        e['p2_b'] = nc.dram_tensor(f"w{k}_p2b", [64, 1], F32, kind="ExternalInput")
        e['p3_w'] = nc.dram_tensor(f"w{k}_p3", [65, 2], BF16, kind="ExternalInput")
        wd.append(e)

    with tile.TileContext(nc) as tc:
        with (
            tc.tile_pool(name="xpool", bufs=1) as xpool,
            tc.tile_pool(name="coord", bufs=1) as cpool,
            tc.tile_pool(name="gpool", bufs=1) as gpool,
            tc.tile_pool(name="wstream", bufs=2) as wpool,
            tc.tile_pool(name="psum", bufs=8, space="PSUM") as psum,
        ):
            ident = cpool.tile([128, 128], F32)
            make_identity(nc, ident[:])

            # persistent activation tensors
            X = [xpool.tile([66, n_inst, HALO], BF16, tag="X0", name="X0")] + [
                xpool.tile([128, n_inst, HALO], BF16, tag=f"X{i}", name=f"X{i}") for i in range(1, 9)]
            P1 = [xpool.tile([128, n_inst, 128], BF16, tag=f"P1_{mo}", name=f"P1_{mo}") for mo in range(2)]
            P2 = xpool.tile([65, n_inst, 128], BF16)
            nc.vector.memset(P2[64:65, :, :], 1.0)
            blend = xpool.tile([128, n_inst, 66], F32)

            # coord tiles
            py = cpool.tile([128, n_inst, 2], F32)
            py_img = cpool.tile([128, n_inst, 2], F32)
            can = cpool.tile([128, n_inst, 2], F32)
            pyneg = cpool.tile([128, n_inst, 2], F32)
            mx = cpool.tile([128, n_inst, 2], F32)
            idx_basef = cpool.tile([128, n_inst], F32)
            W10q = [cpool.tile([128, 2, n_inst // max(1, n_inst // 4), 5], BF16,
                               tag=f"W10q{h}", name=f"W10q{h}")
                    for h in range(max(1, n_inst // 4))]
            idx_f = cpool.tile([128, n_inst * 2], F32)
            idx_i = cpool.tile([128, n_inst * 2], I16)
            idxw = cpool.tile([128, n_inst * 16], I16)
            gmax = [cpool.tile([128, n_inst], F32, tag=f"gmax{mo}", name=f"gmax{mo}") for mo in range(2)]
            gmax_bf = [cpool.tile([128, n_inst], BF16, tag=f"gmaxb{mo}", name=f"gmaxb{mo}") for mo in range(2)]
            vb1 = [cpool.tile([128, n_inst], F32, tag=f"vb1{mo}", name=f"vb1{mo}") for mo in range(2)]

            # scratch for coord math
            def ctile(tag):
                return cpool.tile([128, n_inst], F32, tag=tag, name=tag)

            # per-snake bias/scale tiles (persistent, tiny)
            svec, bsvec, tvec, fusb, p1b, p2b = [], [], [], [], [], []
            for k in range(n_iter):
                s_t = cpool.tile([128, 8], F32, tag=f"s{k}")
                bs_t = cpool.tile([128, 8], F32, tag=f"bs{k}")
                t_t = cpool.tile([128, 8], F32, tag=f"t{k}")
                fb_t = cpool.tile([128, 2], F32, tag=f"fb{k}")
                p1b_t = cpool.tile([128, 2], F32, tag=f"p1b{k}")
                p2b_t = cpool.tile([64, 1], F32, tag=f"p2b{k}")
                nc.sync.dma_start(out=s_t[:], in_=wd[k]['s'][:])
                nc.sync.dma_start(out=bs_t[:], in_=wd[k]['bs'][:])
                nc.sync.dma_start(out=t_t[:], in_=wd[k]['t'][:])
                nc.sync.dma_start(out=fb_t[:], in_=wd[k]['fus_b'][:])
                nc.sync.dma_start(out=p1b_t[:], in_=wd[k]['p1_b'][:])
                nc.sync.dma_start(out=p2b_t[:], in_=wd[k]['p2_b'][:])
                svec.append(s_t); bsvec.append(bs_t); tvec.append(t_t)
                fusb.append(fb_t); p1b.append(p1b_t); p2b.append(p2b_t)

            # init py = clip(polys/4, 0, 127); idx_base = ind*4096
            nc.sync.dma_start(out=py[:], in_=polys_pt[:].rearrange("p (n c) -> p n c", c=2))
            nc.vector.tensor_scalar(py[:], py[:], 0.25, None, AL.mult)
            nc.vector.tensor_scalar(py[:], py[:], 127.0, 0.0, AL.min, AL.max)
            nc.sync.dma_start(out=idx_basef[:], in_=ind_rep[:])
            nc.vector.tensor_scalar(idx_basef[:], idx_basef[:], 4096.0, None, AL.mult)

            feat_win = _ap(feat[:], 0, [[256, FEAT_ROWS], [1, 512]])

            for k in range(n_iter):
                wk = wd[k]
                # ---------------- A: coordinate / weight / index math (PT layout)
                px = _ap(py[:], 0, [[n_inst * 2, 128], [2, n_inst]])
                pyy = _ap(py[:], 1, [[n_inst * 2, 128], [2, n_inst]])

                t_x = ctile("t_x"); x0i = cpool.tile([128, n_inst], I32, tag="x0i", name="x0i")
                x0f = ctile("x0f"); wx1 = ctile("wx1"); xb = ctile("xb")
                vx0 = ctile("vx0"); vx1 = ctile("vx1"); tmp = ctile("tmp")
                aw = ctile("aw"); bw = ctile("bw")
                e_eq = ctile("e_eq"); e_lo = ctile("e_lo"); e_hi = ctile("e_hi")
                ws0 = ctile("ws0"); ws1 = ctile("ws1")
                xq = ctile("xq"); xqi = cpool.tile([128, n_inst], I32, tag="xqi", name="xqi")
                s_off = ctile("s_off")
                m_prev = None

                nc.vector.tensor_scalar(t_x[:], px, 1.0, None, AL.subtract)
                nc.vector.tensor_copy(x0i[:], t_x[:])
                nc.vector.tensor_copy(x0f[:], x0i[:])
                nc.vector.tensor_scalar(t_x[:], px, 0.5, None, AL.subtract)
                nc.vector.tensor_tensor(out=wx1[:], in0=t_x[:], in1=x0f[:], op=AL.subtract)
                nc.vector.tensor_scalar(xb[:], x0f[:], 126.0, 0.0, AL.min, AL.max)
                nc.vector.tensor_scalar(vx0[:], x0f[:], 0.0, None, AL.is_ge)
                nc.vector.tensor_scalar(tmp[:], x0f[:], 127.0, None, AL.is_le)
                nc.vector.tensor_tensor(out=vx0[:], in0=vx0[:], in1=tmp[:], op=AL.mult)
                nc.vector.tensor_scalar(vx1[:], x0f[:], -1.0, None, AL.is_ge)
                nc.vector.tensor_scalar(tmp[:], x0f[:], 126.0, None, AL.is_le)
                nc.vector.tensor_tensor(out=vx1[:], in0=vx1[:], in1=tmp[:], op=AL.mult)
                nc.vector.tensor_scalar(aw[:], wx1[:], -1.0, 1.0, AL.mult, AL.add)
                nc.vector.tensor_tensor(out=aw[:], in0=aw[:], in1=vx0[:], op=AL.mult)
                nc.vector.tensor_tensor(out=bw[:], in0=wx1[:], in1=vx1[:], op=AL.mult)
                nc.vector.tensor_tensor(out=e_eq[:], in0=x0f[:], in1=xb[:], op=AL.is_equal)
                nc.vector.tensor_tensor(out=e_lo[:], in0=x0f[:], in1=xb[:], op=AL.is_lt)
                nc.vector.tensor_tensor(out=e_hi[:], in0=x0f[:], in1=xb[:], op=AL.is_gt)
                nc.vector.tensor_tensor(out=ws0[:], in0=aw[:], in1=e_eq[:], op=AL.mult)
                nc.vector.tensor_tensor(out=tmp[:], in0=bw[:], in1=e_lo[:], op=AL.mult)
                nc.vector.tensor_tensor(out=ws0[:], in0=ws0[:], in1=tmp[:], op=AL.add)
                nc.vector.tensor_tensor(out=ws1[:], in0=bw[:], in1=e_eq[:], op=AL.mult)
                nc.vector.tensor_tensor(out=tmp[:], in0=aw[:], in1=e_hi[:], op=AL.mult)
                nc.vector.tensor_tensor(out=ws1[:], in0=ws1[:], in1=tmp[:], op=AL.add)
                # xq = floor(xb/4) via round((xb-1.5)*0.25); s_off = xb - 4*xq
                nc.vector.tensor_scalar(xq[:], xb[:], 1.5, 0.25, AL.subtract, AL.mult)
                nc.vector.tensor_copy(xqi[:], xq[:])
                nc.vector.tensor_copy(xq[:], xqi[:])
                nc.vector.tensor_scalar(s_off[:], xq[:], -4.0, None, AL.mult)
                nc.vector.tensor_tensor(out=s_off[:], in0=s_off[:], in1=xb[:], op=AL.add)

                # y side
                y0f = ctile("y0f"); wy1 = ctile("wy1")
                yb0 = ctile("yb0"); yb1 = ctile("yb1")
                vy0 = ctile("vy0"); vy1 = ctile("vy1")
                wy0v = ctile("wy0v"); wy1v = ctile("wy1v")
                nc.vector.tensor_scalar(t_x[:], pyy, 1.0, None, AL.subtract)
                nc.vector.tensor_copy(x0i[:], t_x[:])
                nc.vector.tensor_copy(y0f[:], x0i[:])
                nc.vector.tensor_scalar(t_x[:], pyy, 0.5, None, AL.subtract)
                nc.vector.tensor_tensor(out=wy1[:], in0=t_x[:], in1=y0f[:], op=AL.subtract)
                nc.vector.tensor_scalar(yb0[:], y0f[:], 127.0, 0.0, AL.min, AL.max)
                nc.vector.tensor_scalar(yb1[:], y0f[:], 1.0, None, AL.add)
                nc.vector.tensor_scalar(yb1[:], yb1[:], 127.0, 0.0, AL.min, AL.max)
                nc.vector.tensor_scalar(vy0[:], y0f[:], 0.0, None, AL.is_ge)
                nc.vector.tensor_scalar(tmp[:], y0f[:], 127.0, None, AL.is_le)
                nc.vector.tensor_tensor(out=vy0[:], in0=vy0[:], in1=tmp[:], op=AL.mult)
                nc.vector.tensor_scalar(vy1[:], y0f[:], -1.0, None, AL.is_ge)
                nc.vector.tensor_scalar(tmp[:], y0f[:], 126.0, None, AL.is_le)
                nc.vector.tensor_tensor(out=vy1[:], in0=vy1[:], in1=tmp[:], op=AL.mult)
                nc.vector.tensor_scalar(wy0v[:], wy1[:], -1.0, 1.0, AL.mult, AL.add)
                nc.vector.tensor_tensor(out=wy0v[:], in0=wy0v[:], in1=vy0[:], op=AL.mult)
                nc.vector.tensor_tensor(out=wy1v[:], in0=wy1[:], in1=vy1[:], op=AL.mult)

                # W10h[h][p, y, n, s] = wy_yv * (ws0*[s_off==s] + ws1*[s_off==s-1])
                for s in range(5):
                    first = True
                    if s < 4:
                        nc.vector.tensor_scalar(tmp[:], s_off[:], float(s), None, AL.is_equal)
                        nc.vector.tensor_tensor(out=e_eq[:], in0=ws0[:], in1=tmp[:], op=AL.mult)
                        first = False
                    if s >= 1:
                        nc.vector.tensor_scalar(tmp[:], s_off[:], float(s - 1), None, AL.is_equal)
                        nc.vector.tensor_tensor(out=tmp[:], in0=ws1[:], in1=tmp[:], op=AL.mult)
                        if first:
                            nc.vector.tensor_copy(e_eq[:], tmp[:])
                        else:
                            nc.vector.tensor_tensor(out=e_eq[:], in0=e_eq[:], in1=tmp[:], op=AL.add)
                    # e_eq now holds the x-part slot weight for slot s
                    for q in range(NQ):
                        qp = 2 * QI * 5
                        for yi, wyv in enumerate([wy0v, wy1v]):
                            dst = _ap(W10q[q][:], yi * QI * 5 + s, [[qp, 128], [5, QI]])
                            nc.vector.tensor_tensor(out=dst, in0=e_eq[:, q * QI:(q + 1) * QI],
                                                    in1=wyv[:, q * QI:(q + 1) * QI], op=AL.mult)

                # gather indices: idx_y = ind*4096 + yb*32 + xq
                for yi, yb in enumerate([yb0, yb1]):
                    sl = idx_f[:, yi * n_inst:(yi + 1) * n_inst]
                    nc.vector.tensor_scalar(sl, yb[:], 32.0, None, AL.mult)
                    nc.vector.tensor_tensor(out=sl, in0=sl, in1=idx_basef[:], op=AL.add)
                    nc.vector.tensor_tensor(out=sl, in0=sl, in1=xq[:], op=AL.add)
                nc.vector.tensor_copy(idx_i[:], idx_f[:])

                # ---------------- B: index rearrangement roundtrip through DRAM
                nc.sync.dma_start(
                    out=idx_scr[:2 * n_inst * 128].rearrange("(s p) -> p s", p=128),
                    in_=idx_i[:])
                for r in range(8):
                    nc.sync.dma_start(
                        out=idxw[16 * r:16 * (r + 1), :],
                        in_=idx_scr[:2 * n_inst * 128].rearrange("(c r) -> r c", r=16))

                # ---------------- C+D: can (needs gpsimd all-reduce) for blend cols 64:65
                nc.vector.tensor_scalar(pyneg[:], py[:], -1.0, None, AL.mult)
                nc.gpsimd.partition_all_reduce(mx[:], pyneg[:], 128,
                                               bass_isa.ReduceOp.max)
                nc.vector.tensor_tensor(out=can[:], in0=py[:], in1=mx[:], op=AL.add)
                bl_can = _ap(blend[:], 64, [[n_inst * 66, 128], [66, n_inst], [1, 2]])
                nc.vector.tensor_scalar(bl_can, can[:], 4.0, None, AL.mult)

                # ---------------- E: gather + blend per quarter
                for q in range(NQ):
                    g_t = gpool.tile([128, 2, QI, 512], BF16, tag="gq", name="g_t", bufs=2)
                    for yi in range(2):
                        c0 = yi * n_inst * 8 + q * QI * 8
                        nc.gpsimd.dma_gather(
                            g_t[:, yi, :, :], feat_win, idxw[:, c0:c0 + QI * 8],
                            QI * 128, QI * 128, 512, elem_step=256)
                    prod = gpool.tile([128, 2 * QI, 64, 5], BF16, tag="prod", name="prod")
                    gp = 2 * QI * 512
                    # logical order (y-inst merged, ch, slot)
                    in0 = _ap(g_t[:], 0, [[gp, 128], [512, 2 * QI], [1, 64], [64, 5]])
                    in1 = _ap(W10q[q][:], 0, [[2 * QI * 5, 128], [5, 2 * QI], [0, 64], [1, 5]])
                    nc.vector.tensor_tensor(out=prod[:], in0=in0, in1=in1, op=AL.mult)
                    r1 = gpool.tile([128, 2 * QI, 64], F32, tag="r1", name="r1")
                    nc.vector.tensor_reduce(out=r1[:], in_=prod[:], axis=AX.X, op=AL.add)
                    # second reduce over y: view r1 as (inst, ch, y)
                    r1v = _ap(r1[:], 0, [[2 * QI * 64, 128], [64, QI], [1, 64], [QI * 64, 2]])
                    red = _ap(blend[:], q * QI * 66,
                              [[n_inst * 66, 128], [66, QI], [1, 64]])
                    nc.vector.tensor_reduce(out=red, in_=r1v, axis=AX.X, op=AL.add)

                # ---------------- F: transpose blend -> X0 (+ halo)
                for n in range(n_inst):
                    ps_t = psum.tile([66, 128], F32, tag="ps", space="PSUM")
                    nc.tensor.transpose(out=ps_t[:], in_=blend[:, n, :], identity=ident[:])
                    nc.scalar.copy(X[0][:, n, 16:144], ps_t[:])
                for hh in range(NH):
                    isl = slice(hh * HI, (hh + 1) * HI)
                    nc.vector.tensor_copy(X[0][:, isl, 0:16], X[0][:, isl, 128:144])
                    nc.vector.tensor_copy(X[0][:, isl, 144:160], X[0][:, isl, 16:32])

                # ---------------- G: conv blocks
                for blk in range(8):
                    Xin, Xout = X[blk], X[blk + 1]
                    K = 66 if blk == 0 else 128
                    dil = 1 if blk == 0 else DIL[blk - 1]
                    wsrc = wk['head'] if blk == 0 else wk[f'res{blk - 1}']
                    taps = wpool.tile([K, 9, 128], BF16, tag="taps")
                    nc.sync.dma_start(out=taps[:], in_=wsrc[:, :K, :].rearrange("t k m -> k t m"))
                    s_ap = svec[k][:, blk:blk + 1]
                    bs_ap = bsvec[k][:, blk:blk + 1]
                    for h in range(NH):
                        pst = [psum.tile([128, 4 * 128], F32, tag="ps", space="PSUM", name=f"pst{h}_{i}")
                               for i in range(NCH)]
                        tap_order = [4, 5, 6, 7, 8, 3, 2, 1, 0]
                        for ti, t in enumerate(tap_order):
                            sh = 16 + (t - 4) * dil
                            for cch in range(NCH):
                                g0 = h * HI + cch * 4
                                rhs = Xin[:K, g0:g0 + 4, sh:sh + 128]
                                nc.tensor.matmul(pst[cch][:], taps[:, t, :], rhs,
                                                 start=(ti == 0), stop=(ti == 8))
                        for cch in range(NCH):
                            g0 = h * HI + cch * 4
                            octr = Xout[:, g0:g0 + 4, 16:144]
                            nc.scalar.activation(octr, pst[cch][:].rearrange("p (g n) -> p g n", g=4),
                                                 AF.Relu, bias=bs_ap, scale=s_ap)
                            if wmeta[k]['has_t']:
                                nc.vector.tensor_scalar(octr, octr, tvec[k][:, blk:blk + 1],
                                                        None, AL.add)
                            if blk > 0:
                                nc.vector.tensor_tensor(out=octr, in0=octr,
                                                        in1=Xin[:, g0:g0 + 4, 16:144], op=AL.add)
                    for hh in range(NH):
                        isl = slice(hh * HI, (hh + 1) * HI)
                        nc.vector.tensor_copy(Xout[:, isl, 0:16], Xout[:, isl, 128:144])
                        nc.vector.tensor_copy(Xout[:, isl, 144:160], Xout[:, isl, 16:32])

                # ---------------- H: fusion conv + global max
                fusw = wpool.tile([128, 8, 256], BF16, tag="fus")
                nc.sync.dma_start(out=fusw[:], in_=wk['fus_w'][:].rearrange("s k m -> k s m"))
                for h in range(NH):
                    for cch in range(NCH):
                        g0 = h * HI + cch * 4
                        for mo in range(2):
                            pf = psum.tile([128, 4 * 128], F32, tag="ps", space="PSUM")
                            for s in range(8):
                                rhs = X[s + 1][:, g0:g0 + 4, 16:144]
                                nc.tensor.matmul(pf[:], fusw[:, s, mo * 128:(mo + 1) * 128],
                                                 rhs, start=(s == 0), stop=(s == 7))
                            nc.vector.tensor_reduce(
                                out=gmax[mo][:, g0:g0 + 4],
                                in_=pf[:].rearrange("p (g n) -> p g n", g=4),
                                axis=AX.X, op=AL.max)
                for mo in range(2):
                    nc.vector.tensor_scalar(gmax[mo][:], gmax[mo][:],
                                            fusb[k][:, mo:mo + 1], None, AL.add)
                    nc.vector.tensor_copy(gmax_bf[mo][:], gmax[mo][:])

                # v = W1a @ g + b1  (per-instance pred1 bias)
                p1aw = wpool.tile([128, 2, 256], BF16, tag="p1a")
                nc.sync.dma_start(out=p1aw[:], in_=wk['p1a_w'][:].rearrange("s k m -> k s m"))
                for mo in range(2):
                    vps = psum.tile([128, n_inst], F32, tag="ps", space="PSUM")
                    for ki in range(2):
                        nc.tensor.matmul(vps[:], p1aw[:, ki, mo * 128:(mo + 1) * 128],
                                         gmax_bf[ki][:], start=(ki == 0), stop=(ki == 1))
                    nc.vector.tensor_scalar(vb1[mo][:], vps[:], p1b[k][:, mo:mo + 1],
                                            None, AL.add)

                # ---------------- I: pred1
                p1w = wpool.tile([128, 8, 256], BF16, tag="p1")
                nc.sync.dma_start(out=p1w[:], in_=wk['p1_w'][:].rearrange("s k m -> k s m"))
                for h in range(NH):
                    for cch in range(NCH):
                        g0 = h * HI + cch * 4
                        for mo in range(2):
                            pp = psum.tile([128, 4 * 128], F32, tag="ps", space="PSUM")
                            for s in range(8):
                                rhs = X[s + 1][:, g0:g0 + 4, 16:144]
                                nc.tensor.matmul(pp[:], p1w[:, s, mo * 128:(mo + 1) * 128],
                                                 rhs, start=(s == 0), stop=(s == 7))
                            for j in range(4):
                                n = g0 + j
                                nc.scalar.activation(
                                    P1[mo][:, n, :], pp[:, j * 128:(j + 1) * 128],
                                    AF.Relu, bias=vb1[mo][:, n:n + 1], scale=1.0)

                # ---------------- J: pred2
                p2w = wpool.tile([128, 2, 64], BF16, tag="p2")
                nc.sync.dma_start(out=p2w[:], in_=wk['p2_w'][:].rearrange("s k m -> k s m"))
                for h in range(NH):
                    for cch in range(NCH):
                        g0 = h * HI + cch * 4
                        pp = psum.tile([64, 4 * 128], F32, tag="ps", space="PSUM")
                        for ki in range(2):
                            nc.tensor.matmul(pp[:], p2w[:, ki, :], P1[ki][:, g0:g0 + 4, :],
                                             start=(ki == 0), stop=(ki == 1))
                        nc.scalar.activation(P2[:64, g0:g0 + 4, :],
                                             pp[:].rearrange("p (g n) -> p g n", g=4),
                                             AF.Relu, bias=p2b[k][:, 0:1], scale=1.0)

                # ---------------- K: pred3 (per-instance lhsT) + py update
                p3w = wpool.tile([65, 2], BF16, tag="p3")
                nc.sync.dma_start(out=p3w[:], in_=wk['p3_w'][:])
                p3ps = psum.tile([128, n_inst * 2], F32, tag="ps", space="PSUM")
                for n in range(n_inst):
                    nc.tensor.matmul(p3ps[:, 2 * n:2 * n + 2], P2[:, n, :], p3w[:],
                                     start=True, stop=True, skip_group_check=True)
                nc.vector.tensor_scalar(py_img[:], py[:], 4.0, None, AL.mult)
                nc.vector.tensor_tensor(
                    out=py_img[:], in0=py_img[:],
                    in1=p3ps[:].rearrange("p (n c) -> p n c", c=2), op=AL.add)
                nc.sync.dma_start(
                    out=out_pys[k].rearrange("p (n c) -> p n c", c=2), in_=py_img[:])
                if k + 1 < n_iter:
                    nc.vector.tensor_scalar(py[:], py_img[:], 0.25, None, AL.mult)

    nc.compile()
    return nc


_PROGRAM_CACHE = {}


def kernel(cnn_feature, polys, ind, snake_params):
    cnn_feature = np.asarray(cnn_feature, np.float32)
    polys = np.asarray(polys, np.float32)
    ind = np.asarray(ind)
    n_total, n_pts = polys.shape[0], polys.shape[1]
    assert n_pts == PTS and cnn_feature.shape == (B, C, H, W)
    n_inst = n_total // N_CORES
    n_iter = len(snake_params)

    wmeta = prep_weights(snake_params)

    key = (n_inst, n_iter)
    if key not in _PROGRAM_CACHE:
        _PROGRAM_CACHE[key] = build_program(n_inst, n_iter, wmeta)
    nc = _PROGRAM_CACHE[key]

    # host data prep
    feat_t = np.ascontiguousarray(cnn_feature.transpose(0, 2, 3, 1)).reshape(-1, 64)
    feat_pad = np.zeros((FEAT_ROWS * 256 + 512,), bf16)
    feat_pad[:feat_t.size] = feat_t.reshape(-1).astype(bf16)

    wmap = {}
    for k in range(n_iter):
        m = wmeta[k]
        wmap[f"w{k}_head"] = np.zeros((9, 66, 128), bf16)
        wmap[f"w{k}_head"][:, :66, :] = m['taps'][0]
        for i in range(7):
            wmap[f"w{k}_res{i}"] = m['taps'][i + 1]
        wmap[f"w{k}_s"] = m['s']; wmap[f"w{k}_bs"] = m['bs']; wmap[f"w{k}_t"] = m['t']
        wmap[f"w{k}_fus"] = m['fus_w']; wmap[f"w{k}_fusb"] = m['fus_b']
        wmap[f"w{k}_p1"] = m['p1_w']; wmap[f"w{k}_p1a"] = m['p1a_w']
        wmap[f"w{k}_p1b"] = m['p1_b']
        wmap[f"w{k}_p2"] = m['p2_w']; wmap[f"w{k}_p2b"] = m['p2_b']
        wmap[f"w{k}_p3"] = m['p3_w']

    in_maps = []
    for c in range(N_CORES):
        sl = slice(c * n_inst, (c + 1) * n_inst)
        pp = polys[sl]  # [n_inst, 128, 2]
        polys_pt = np.ascontiguousarray(pp.transpose(1, 0, 2)).reshape(PTS, n_inst * 2)
        ind_rep = np.broadcast_to(ind[sl].astype(np.float32)[None, :],
                                  (PTS, n_inst)).copy()
        m = {"feat": feat_pad, "polys_pt": polys_pt, "ind_rep": ind_rep}
        m.update(wmap)
        in_maps.append(m)

    res = run_bass_kernel_spmd(nc, in_maps, core_ids=list(range(N_CORES)))
    outs = []
    for c in range(N_CORES):
        o = res.results[c]["out_pys"]  # [n_iter, 128, n_inst*2]
        outs.append(o.reshape(n_iter, PTS, n_inst, 2).transpose(0, 2, 1, 3))
    return np.concatenate(outs, axis=1).astype(np.float32)  # [n_iter, N, P, 2]


# revision 15
# speedup vs baseline: 86.6501x; 1.0042x over previous
"""Trainium2 Bass kernel for the 3-iteration Snake/deep-snake GNN head.

Strategy: pure data-parallel over the 256 polygon instances (32 per core x 8
cores).  Per core, one fully-unrolled program runs all 3 snake iterations:

  - bilinear gather: cnn_feature is transposed to channel-last on the host;
    on device a single int16-indexed dma_gather per (y-row-set, quarter)
    fetches 8-pixel windows at 4-pixel granularity (global row index
    ind*4096 + y*32 + floor(xb/4) <= 32767 fits int16).
  - the 2x2 bilinear blend is one 5-dim tensor_tensor multiply against
    per-(point,instance) weights + a reduce over (y,slot).
  - circular dilated convs run as 9 shifted matmuls over a halo'd
    [C, inst*160] activation tensor, batched 4 instances per N=512 matmul,
    accumulating in PSUM; BN (eval mode) folds into the ACT relu epilogue.
  - fusion / pred 1x1 convs are plain K-tiled matmuls; the global-max path
    becomes a free-dim reduce_max plus a tiny matmul whose result enters the
    pred1 epilogue as a per-instance ACT bias.
"""

import numpy as np
import ml_dtypes

import concourse.bass as bass
import concourse.bass_isa as bass_isa
import concourse.bacc as bacc
import concourse.mybir as mybir
import concourse.tile as tile
from concourse.bass_utils import run_bass_kernel_spmd
from concourse.masks import make_identity

F32 = mybir.dt.float32
BF16 = mybir.dt.bfloat16
I16 = mybir.dt.int16
I32 = mybir.dt.int32
AL = mybir.AluOpType
AF = mybir.ActivationFunctionType
AX = mybir.AxisListType

B, C, H, W = 8, 64, 128, 128
PTS = 128
RO = 4.0
DIL = [1, 1, 1, 2, 2, 4, 4]
BN_EPS = 1e-5
N_CORES = 8
HALO = PTS + 32  # 16 halo points each side (max dilation 4 * 4 taps)
FEAT_ROWS = B * H * W // 4  # 4-pixel granule rows = 32768

bf16 = ml_dtypes.bfloat16


def _ap(t_ap, extra_off, dims):
    return bass.AP(t_ap.tensor, t_ap.offset + extra_off, dims)


def prep_weights(snake_params):
    """Per snake: transpose conv weights to lhsT layout, fold BN."""
    out = []
    for p in snake_params:
        d = {}
        hw, hb, hg, hbeta, hm, hv = p['head']
        blocks = [(hw, hb, hg, hbeta, hm, hv)] + [tuple(bp) for bp in p['res']]
        tap_ws, s_vecs, bs_vecs, t_vecs = [], [], [], []
        for (w, b, g, beta, m, v) in blocks:
            w = np.asarray(w, np.float32)
            s = np.asarray(g, np.float32) / np.sqrt(np.asarray(v, np.float32) + BN_EPS)
            assert (s > 0).all(), "BN scale must be positive for folded relu"
            tap_ws.append(np.ascontiguousarray(w.transpose(2, 1, 0)).astype(bf16))  # [9, cin, cout]
            s_vecs.append(s)
            bs_vecs.append(np.asarray(b, np.float32) * s)
            t_vecs.append(np.asarray(beta, np.float32) - np.asarray(m, np.float32) * s)
        d['taps'] = tap_ws                      # [0]=head [9,66,128]; 1..7 res [9,128,128]
        d['s'] = np.stack(s_vecs, 1)            # [128, 8]
        d['bs'] = np.stack(bs_vecs, 1)          # [128, 8]
        d['t'] = np.stack(t_vecs, 1)            # [128, 8]
        d['has_t'] = bool(np.any(d['t']))
        fw, fb = p['fusion']
        fw = np.asarray(fw, np.float32)[:, :, 0]           # [256, 1024]
        d['fus_w'] = np.ascontiguousarray(
            fw.T.reshape(8, 128, 256)).astype(bf16)        # [8 ktile, 128, 256]
        d['fus_b'] = np.asarray(fb, np.float32).reshape(2, 128).T.copy()  # [128, 2]
        (w1, b1), (w2, b2), (w3, b3) = p['pred']
        w1 = np.asarray(w1, np.float32)[:, :, 0]           # [256, 1280]
        d['p1_w'] = np.ascontiguousarray(
            w1[:, 256:].T.reshape(8, 128, 256)).astype(bf16)
        d['p1a_w'] = np.ascontiguousarray(
            w1[:, :256].T.reshape(2, 128, 256)).astype(bf16)
        d['p1_b'] = np.asarray(b1, np.float32).reshape(2, 128).T.copy()   # [128, 2]
        w2 = np.asarray(w2, np.float32)[:, :, 0]           # [64, 256]
        d['p2_w'] = np.ascontiguousarray(w2.T.reshape(2, 128, 64)).astype(bf16)
        d['p2_b'] = np.asarray(b2, np.float32).reshape(64, 1).copy()      # [64, 1]
        w3 = np.asarray(w3, np.float32)[:, :, 0]           # [2, 64]
        d['p3_w'] = np.concatenate(
            [w3.T, np.asarray(b3, np.float32)[None, :]], 0).astype(bf16)  # [65, 2]
        out.append(d)
    return out


def build_program(n_inst, n_iter, wmeta):
    """Build the per-core Bass program.  n_inst instances, n_iter snake iters."""
    assert n_inst % 4 == 0
    NQ = max(1, n_inst // 4)         # gather groups
    QI = n_inst // NQ                # instances per gather group (4)
    NH = max(1, n_inst // 16)        # psum halves
    HI = n_inst // NH                # instances per half (16)
    NCH = HI // 4                    # 4-instance chunks per half

    nc = bacc.Bacc("TRN2", target_bir_lowering=False, debug=False,
                   num_devices=N_CORES, name="snake")
    feat = nc.dram_tensor("feat", [FEAT_ROWS * 256 + 512], BF16, kind="ExternalInput")
    polys_pt = nc.dram_tensor("polys_pt", [PTS, n_inst * 2], F32, kind="ExternalInput")
    ind_rep = nc.dram_tensor("ind_rep", [PTS, n_inst], F32, kind="ExternalInput")
    out_pys = nc.dram_tensor("out_pys", [n_iter, PTS, n_inst * 2], F32,
                             kind="ExternalOutput")
    idx_scr = nc.dram_tensor("idx_scr", [2 * n_inst * PTS], I16, kind="Internal")

    wd = []  # per-snake dram tensors
    for k in range(n_iter):
        m = wmeta[k]
        e = {}
        e['head'] = nc.dram_tensor(f"w{k}_head", [9, 66, 128], BF16, kind="ExternalInput")
        for i in range(7):
            e[f'res{i}'] = nc.dram_tensor(f"w{k}_res{i}", [9, 128, 128], BF16, kind="ExternalInput")
        e['s'] = nc.dram_tensor(f"w{k}_s", [128, 8], F32, kind="ExternalInput")
        e['bs'] = nc.dram_tensor(f"w{k}_bs", [128, 8], F32, kind="ExternalInput")
        e['t'] = nc.dram_tensor(f"w{k}_t", [128, 8], F32, kind="ExternalInput")
        e['fus_w'] = nc.dram_tensor(f"w{k}_fus", [8, 128, 256], BF16, kind="ExternalInput")
        e['fus_b'] = nc.dram_tensor(f"w{k}_fusb", [128, 2], F32, kind="ExternalInput")
        e['p1_w'] = nc.dram_tensor(f"w{k}_p1", [8, 128, 256], BF16, kind="ExternalInput")
        e['p1a_w'] = nc.dram_tensor(f"w{k}_p1a", [2, 128, 256], BF16, kind="ExternalInput")
        e['p1_b'] = nc.dram_tensor(f"w{k}_p1b", [128, 2], F32, kind="ExternalInput")
        e['p2_w'] = nc.dram_tensor(f"w{k}_p2", [2, 128, 64], BF16, kind="ExternalInput")
        e['p2_b'] = nc.dram_tensor(f"w{k}_p2b", [64, 1], F32, kind="ExternalInput")
        e['p3_w'] = nc.dram_tensor(f"w{k}_p3", [65, 2], BF16, kind="ExternalInput")
        wd.append(e)

    with tile.TileContext(nc) as tc:
        with (
            tc.tile_pool(name="xpool", bufs=1) as xpool,
            tc.tile_pool(name="coord", bufs=1) as cpool,
            tc.tile_pool(name="gpool", bufs=1) as gpool,
            tc.tile_pool(name="wstream", bufs=2) as wpool,
            tc.tile_pool(name="psum", bufs=8, space="PSUM") as psum,
        ):
            ident = cpool.tile([128, 128], F32)
            make_identity(nc, ident[:])

            # persistent activation tensors
            X = [xpool.tile([66, n_inst, HALO], BF16, tag="X0", name="X0")] + [
                xpool.tile([128, n_inst, HALO], BF16, tag=f"X{i}", name=f"X{i}") for i in range(1, 9)]
            P1 = [xpool.tile([128, n_inst, 128], BF16, tag=f"P1_{mo}", name=f"P1_{mo}") for mo in range(2)]
            P2 = xpool.tile([65, n_inst, 128], BF16)
            nc.vector.memset(P2[64:65, :, :], 1.0)
            blend = xpool.tile([128, n_inst, 66], F32)

            # coord tiles
            py = cpool.tile([128, n_inst, 2], F32)
            py_img = cpool.tile([128, n_inst, 2], F32)
            can = cpool.tile([128, n_inst, 2], F32)
            pyneg = cpool.tile([128, n_inst, 2], F32)
            mx = cpool.tile([128, n_inst, 2], F32)
            idx_basef = cpool.tile([128, n_inst], F32)
            W10q = [cpool.tile([128, 2, n_inst // max(1, n_inst // 4), 5], BF16,
                               tag=f"W10q{h}", name=f"W10q{h}")
                    for h in range(max(1, n_inst // 4))]
            idx_f = cpool.tile([128, n_inst * 2], F32)
            idx_i = cpool.tile([128, n_inst * 2], I16)
            idxw = cpool.tile([128, n_inst * 16], I16)
            gmax = [cpool.tile([128, n_inst], F32, tag=f"gmax{mo}", name=f"gmax{mo}") for mo in range(2)]
            gmax_bf = [cpool.tile([128, n_inst], BF16, tag=f"gmaxb{mo}", name=f"gmaxb{mo}") for mo in range(2)]
            vb1 = [cpool.tile([128, n_inst], F32, tag=f"vb1{mo}", name=f"vb1{mo}") for mo in range(2)]

            # scratch for coord math
            def ctile(tag):
                return cpool.tile([128, n_inst], F32, tag=tag, name=tag)

            # per-snake bias/scale tiles (persistent, tiny)
            svec, bsvec, tvec, fusb, p1b, p2b = [], [], [], [], [], []
            for k in range(n_iter):
                s_t = cpool.tile([128, 8], F32, tag=f"s{k}")
                bs_t = cpool.tile([128, 8], F32, tag=f"bs{k}")
                t_t = cpool.tile([128, 8], F32, tag=f"t{k}")
                fb_t = cpool.tile([128, 2], F32, tag=f"fb{k}")
                p1b_t = cpool.tile([128, 2], F32, tag=f"p1b{k}")
                p2b_t = cpool.tile([64, 1], F32, tag=f"p2b{k}")
                nc.sync.dma_start(out=s_t[:], in_=wd[k]['s'][:])
                nc.sync.dma_start(out=bs_t[:], in_=wd[k]['bs'][:])
                nc.sync.dma_start(out=t_t[:], in_=wd[k]['t'][:])
                nc.sync.dma_start(out=fb_t[:], in_=wd[k]['fus_b'][:])
                nc.sync.dma_start(out=p1b_t[:], in_=wd[k]['p1_b'][:])
                nc.sync.dma_start(out=p2b_t[:], in_=wd[k]['p2_b'][:])
                svec.append(s_t); bsvec.append(bs_t); tvec.append(t_t)
                fusb.append(fb_t); p1b.append(p1b_t); p2b.append(p2b_t)

            # init py = clip(polys/4, 0, 127); idx_base = ind*4096
            nc.sync.dma_start(out=py[:], in_=polys_pt[:].rearrange("p (n c) -> p n c", c=2))
            nc.vector.tensor_scalar(py[:], py[:], 0.25, None, AL.mult)
            nc.vector.tensor_scalar(py[:], py[:], 127.0, 0.0, AL.min, AL.max)
            nc.sync.dma_start(out=idx_basef[:], in_=ind_rep[:])
            nc.vector.tensor_scalar(idx_basef[:], idx_basef[:], 4096.0, None, AL.mult)

            feat_win = _ap(feat[:], 0, [[256, FEAT_ROWS], [1, 512]])

            for k in range(n_iter):
                wk = wd[k]
                # ---------------- A: index math first (so DMAs/gathers start early)
                px = _ap(py[:], 0, [[n_inst * 2, 128], [2, n_inst]])
                pyy = _ap(py[:], 1, [[n_inst * 2, 128], [2, n_inst]])

                t_x = ctile("t_x"); x0i = cpool.tile([128, n_inst], I32, tag="x0i", name="x0i")
                x0f = ctile("x0f"); wx1 = ctile("wx1"); xb = ctile("xb")
                vx0 = ctile("vx0"); vx1 = ctile("vx1"); tmp = ctile("tmp")
                aw = ctile("aw"); bw = ctile("bw")
                e_eq = ctile("e_eq"); e_lo = ctile("e_lo"); e_hi = ctile("e_hi")
                ws0 = ctile("ws0"); ws1 = ctile("ws1")
                xq = ctile("xq"); xqi = cpool.tile([128, n_inst], I32, tag="xqi", name="xqi")
                s_off = ctile("s_off")
                y0f = ctile("y0f"); wy1 = ctile("wy1")
                yb0 = ctile("yb0"); yb1 = ctile("yb1")
                vy0 = ctile("vy0"); vy1 = ctile("vy1")
                wy0v = ctile("wy0v"); wy1v = ctile("wy1v")

                # x core: floor, base, 4-px granule
                nc.vector.tensor_scalar(t_x[:], px, 1.0, None, AL.subtract)
                nc.vector.tensor_copy(x0i[:], t_x[:])
                nc.vector.tensor_copy(x0f[:], x0i[:])
                nc.vector.tensor_scalar(xb[:], x0f[:], 126.0, 0.0, AL.min, AL.max)
                nc.vector.tensor_scalar(xq[:], xb[:], 1.5, 0.25, AL.subtract, AL.mult)
                nc.vector.tensor_copy(xqi[:], xq[:])
                nc.vector.tensor_copy(xq[:], xqi[:])
                # y core: floor + row clips
                nc.vector.tensor_scalar(tmp[:], pyy, 1.0, None, AL.subtract)
                nc.vector.tensor_copy(xqi[:], tmp[:])
                nc.vector.tensor_copy(y0f[:], xqi[:])
                nc.vector.tensor_scalar(yb0[:], y0f[:], 127.0, 0.0, AL.min, AL.max)
                nc.vector.tensor_scalar(yb1[:], y0f[:], 1.0, None, AL.add)
                nc.vector.tensor_scalar(yb1[:], yb1[:], 127.0, 0.0, AL.min, AL.max)
                # gather indices: idx_y = ind*4096 + yb*32 + xq  (issue ASAP)
                for yi, yb in enumerate([yb0, yb1]):
                    sl = idx_f[:, yi * n_inst:(yi + 1) * n_inst]
                    nc.vector.tensor_scalar(sl, yb[:], 32.0, None, AL.mult)
                    nc.vector.tensor_tensor(out=sl, in0=sl, in1=idx_basef[:], op=AL.add)
                    nc.vector.tensor_tensor(out=sl, in0=sl, in1=xq[:], op=AL.add)
                nc.vector.tensor_copy(idx_i[:], idx_f[:])

                # ---------------- B: index rearrangement roundtrip through DRAM
                nc.sync.dma_start(
                    out=idx_scr[:2 * n_inst * 128].rearrange("(s p) -> p s", p=128),
                    in_=idx_i[:])
                for r in range(8):
                    nc.sync.dma_start(
                        out=idxw[16 * r:16 * (r + 1), :],
                        in_=idx_scr[:2 * n_inst * 128].rearrange("(c r) -> r c", r=16))

                # ---------------- A2: blend-weight math (overlaps gathers)
                nc.vector.tensor_scalar(t_x[:], px, 0.5, None, AL.subtract)
                nc.vector.tensor_tensor(out=wx1[:], in0=t_x[:], in1=x0f[:], op=AL.subtract)
                nc.vector.tensor_scalar(vx0[:], x0f[:], 0.0, None, AL.is_ge)
                nc.vector.tensor_scalar(tmp[:], x0f[:], 127.0, None, AL.is_le)
                nc.vector.tensor_tensor(out=vx0[:], in0=vx0[:], in1=tmp[:], op=AL.mult)
                nc.vector.tensor_scalar(vx1[:], x0f[:], -1.0, None, AL.is_ge)
                nc.vector.tensor_scalar(tmp[:], x0f[:], 126.0, None, AL.is_le)
                nc.vector.tensor_tensor(out=vx1[:], in0=vx1[:], in1=tmp[:], op=AL.mult)
                nc.vector.tensor_scalar(aw[:], wx1[:], -1.0, 1.0, AL.mult, AL.add)
                nc.vector.tensor_tensor(out=aw[:], in0=aw[:], in1=vx0[:], op=AL.mult)
                nc.vector.tensor_tensor(out=bw[:], in0=wx1[:], in1=vx1[:], op=AL.mult)
                nc.vector.tensor_tensor(out=e_eq[:], in0=x0f[:], in1=xb[:], op=AL.is_equal)
                nc.vector.tensor_tensor(out=e_lo[:], in0=x0f[:], in1=xb[:], op=AL.is_lt)
                nc.vector.tensor_tensor(out=e_hi[:], in0=x0f[:], in1=xb[:], op=AL.is_gt)
                nc.vector.tensor_tensor(out=ws0[:], in0=aw[:], in1=e_eq[:], op=AL.mult)
                nc.vector.tensor_tensor(out=tmp[:], in0=bw[:], in1=e_lo[:], op=AL.mult)
                nc.vector.tensor_tensor(out=ws0[:], in0=ws0[:], in1=tmp[:], op=AL.add)
                nc.vector.tensor_tensor(out=ws1[:], in0=bw[:], in1=e_eq[:], op=AL.mult)
                nc.vector.tensor_tensor(out=tmp[:], in0=aw[:], in1=e_hi[:], op=AL.mult)
                nc.vector.tensor_tensor(out=ws1[:], in0=ws1[:], in1=tmp[:], op=AL.add)
                nc.vector.tensor_scalar(s_off[:], xq[:], -4.0, None, AL.mult)
                nc.vector.tensor_tensor(out=s_off[:], in0=s_off[:], in1=xb[:], op=AL.add)
                nc.vector.tensor_scalar(t_x[:], pyy, 0.5, None, AL.subtract)
                nc.vector.tensor_tensor(out=wy1[:], in0=t_x[:], in1=y0f[:], op=AL.subtract)
                nc.vector.tensor_scalar(vy0[:], y0f[:], 0.0, None, AL.is_ge)
                nc.vector.tensor_scalar(tmp[:], y0f[:], 127.0, None, AL.is_le)
                nc.vector.tensor_tensor(out=vy0[:], in0=vy0[:], in1=tmp[:], op=AL.mult)
                nc.vector.tensor_scalar(vy1[:], y0f[:], -1.0, None, AL.is_ge)
                nc.vector.tensor_scalar(tmp[:], y0f[:], 126.0, None, AL.is_le)
                nc.vector.tensor_tensor(out=vy1[:], in0=vy1[:], in1=tmp[:], op=AL.mult)
                nc.vector.tensor_scalar(wy0v[:], wy1[:], -1.0, 1.0, AL.mult, AL.add)
                nc.vector.tensor_tensor(out=wy0v[:], in0=wy0v[:], in1=vy0[:], op=AL.mult)
                nc.vector.tensor_tensor(out=wy1v[:], in0=wy1[:], in1=vy1[:], op=AL.mult)

                # W10q[q][p, y, n, s] = wy_yv * (ws0*[s_off==s] + ws1*[s_off==s-1])
                for s in range(5):
                    first = True
                    if s < 4:
                        nc.vector.tensor_scalar(tmp[:], s_off[:], float(s), None, AL.is_equal)
                        nc.vector.tensor_tensor(out=e_eq[:], in0=ws0[:], in1=tmp[:], op=AL.mult)
                        first = False
                    if s >= 1:
                        nc.vector.tensor_scalar(tmp[:], s_off[:], float(s - 1), None, AL.is_equal)
                        nc.vector.tensor_tensor(out=tmp[:], in0=ws1[:], in1=tmp[:], op=AL.mult)
                        if first:
                            nc.vector.tensor_copy(e_eq[:], tmp[:])
                        else:
                            nc.vector.tensor_tensor(out=e_eq[:], in0=e_eq[:], in1=tmp[:], op=AL.add)
                    # e_eq now holds the x-part slot weight for slot s
                    for q in range(NQ):
                        qp = 2 * QI * 5
                        for yi, wyv in enumerate([wy0v, wy1v]):
                            dst = _ap(W10q[q][:], yi * QI * 5 + s, [[qp, 128], [5, QI]])
                            nc.vector.tensor_tensor(out=dst, in0=e_eq[:, q * QI:(q + 1) * QI],
                                                    in1=wyv[:, q * QI:(q + 1) * QI], op=AL.mult)

                # ---------------- C+D: can (needs gpsimd all-reduce) for blend cols 64:65
                nc.vector.tensor_scalar(pyneg[:], py[:], -1.0, None, AL.mult)
                nc.gpsimd.partition_all_reduce(mx[:], pyneg[:], 128,
                                               bass_isa.ReduceOp.max)
                nc.vector.tensor_tensor(out=can[:], in0=py[:], in1=mx[:], op=AL.add)
                bl_can = _ap(blend[:], 64, [[n_inst * 66, 128], [66, n_inst], [1, 2]])
                nc.vector.tensor_scalar(bl_can, can[:], 4.0, None, AL.mult)

                # ---------------- E: gather + blend per quarter
                for q in range(NQ):
                    g_t = gpool.tile([128, 2, QI, 512], BF16, tag="gq", name="g_t", bufs=2)
                    for yi in range(2):
                        c0 = yi * n_inst * 8 + q * QI * 8
                        nc.gpsimd.dma_gather(
                            g_t[:, yi, :, :], feat_win, idxw[:, c0:c0 + QI * 8],
                            QI * 128, QI * 128, 512, elem_step=256)
                    prod = gpool.tile([128, 2 * QI, 64, 5], BF16, tag="prod", name="prod")
                    gp = 2 * QI * 512
                    # logical order (y-inst merged, ch, slot)
                    in0 = _ap(g_t[:], 0, [[gp, 128], [512, 2 * QI], [1, 64], [64, 5]])
                    in1 = _ap(W10q[q][:], 0, [[2 * QI * 5, 128], [5, 2 * QI], [0, 64], [1, 5]])
                    nc.vector.tensor_tensor(out=prod[:], in0=in0, in1=in1, op=AL.mult)
                    r1 = gpool.tile([128, 2 * QI, 64], F32, tag="r1", name="r1")
                    nc.vector.tensor_reduce(out=r1[:], in_=prod[:], axis=AX.X, op=AL.add)
                    # second reduce over y: view r1 as (inst, ch, y)
                    r1v = _ap(r1[:], 0, [[2 * QI * 64, 128], [64, QI], [1, 64], [QI * 64, 2]])
                    red = _ap(blend[:], q * QI * 66,
                              [[n_inst * 66, 128], [66, QI], [1, 64]])
                    nc.vector.tensor_reduce(out=red, in_=r1v, axis=AX.X, op=AL.add)

                # ---------------- F: transpose blend -> X0 (+ halo)
                for n in range(n_inst):
                    ps_t = psum.tile([66, 128], F32, tag="ps", space="PSUM")
                    nc.tensor.transpose(out=ps_t[:], in_=blend[:, n, :], identity=ident[:])
                    nc.scalar.copy(X[0][:, n, 16:144], ps_t[:])
                for hh in range(NH):
                    isl = slice(hh * HI, (hh + 1) * HI)
                    nc.vector.tensor_copy(X[0][:, isl, 0:16], X[0][:, isl, 128:144])
                    nc.vector.tensor_copy(X[0][:, isl, 144:160], X[0][:, isl, 16:32])

                # ---------------- G: conv blocks
                for blk in range(8):
                    Xin, Xout = X[blk], X[blk + 1]
                    K = 66 if blk == 0 else 128
                    dil = 1 if blk == 0 else DIL[blk - 1]
                    wsrc = wk['head'] if blk == 0 else wk[f'res{blk - 1}']
                    taps = wpool.tile([K, 9, 128], BF16, tag="taps")
                    nc.sync.dma_start(out=taps[:], in_=wsrc[:, :K, :].rearrange("t k m -> k t m"))
                    s_ap = svec[k][:, blk:blk + 1]
                    bs_ap = bsvec[k][:, blk:blk + 1]
                    for h in range(NH):
                        pst = [psum.tile([128, 4 * 128], F32, tag="ps", space="PSUM", name=f"pst{h}_{i}")
                               for i in range(NCH)]
                        tap_order = [4, 5, 6, 7, 8, 3, 2, 1, 0]
                        for ti, t in enumerate(tap_order):
                            sh = 16 + (t - 4) * dil
                            for cch in range(NCH):
                                g0 = h * HI + cch * 4
                                rhs = Xin[:K, g0:g0 + 4, sh:sh + 128]
                                nc.tensor.matmul(pst[cch][:], taps[:, t, :], rhs,
                                                 start=(ti == 0), stop=(ti == 8))
                        for cch in range(NCH):
                            g0 = h * HI + cch * 4
                            octr = Xout[:, g0:g0 + 4, 16:144]
                            nc.scalar.activation(octr, pst[cch][:].rearrange("p (g n) -> p g n", g=4),
                                                 AF.Relu, bias=bs_ap, scale=s_ap)
                            if wmeta[k]['has_t']:
                                nc.vector.tensor_scalar(octr, octr, tvec[k][:, blk:blk + 1],
                                                        None, AL.add)
                            if blk > 0:
                                nc.vector.tensor_tensor(out=octr, in0=octr,
                                                        in1=Xin[:, g0:g0 + 4, 16:144], op=AL.add)
                    for hh in range(NH):
                        isl = slice(hh * HI, (hh + 1) * HI)
                        nc.vector.tensor_copy(Xout[:, isl, 0:16], Xout[:, isl, 128:144])
                        nc.vector.tensor_copy(Xout[:, isl, 144:160], Xout[:, isl, 16:32])

                # ---------------- H: fusion conv + global max
                fusw = wpool.tile([128, 8, 256], BF16, tag="fus")
                nc.sync.dma_start(out=fusw[:], in_=wk['fus_w'][:].rearrange("s k m -> k s m"))
                for h in range(NH):
                    for cch in range(NCH):
                        g0 = h * HI + cch * 4
                        for mo in range(2):
                            pf = psum.tile([128, 4 * 128], F32, tag="ps", space="PSUM")
                            for s in range(8):
                                rhs = X[s + 1][:, g0:g0 + 4, 16:144]
                                nc.tensor.matmul(pf[:], fusw[:, s, mo * 128:(mo + 1) * 128],
                                                 rhs, start=(s == 0), stop=(s == 7))
                            nc.vector.tensor_reduce(
                                out=gmax[mo][:, g0:g0 + 4],
                                in_=pf[:].rearrange("p (g n) -> p g n", g=4),
                                axis=AX.X, op=AL.max)
                for mo in range(2):
                    nc.vector.tensor_scalar(gmax[mo][:], gmax[mo][:],
                                            fusb[k][:, mo:mo + 1], None, AL.add)
                    nc.vector.tensor_copy(gmax_bf[mo][:], gmax[mo][:])

                # v = W1a @ g + b1  (per-instance pred1 bias)
                p1aw = wpool.tile([128, 2, 256], BF16, tag="p1a")
                nc.sync.dma_start(out=p1aw[:], in_=wk['p1a_w'][:].rearrange("s k m -> k s m"))
                for mo in range(2):
                    vps = psum.tile([128, n_inst], F32, tag="ps", space="PSUM")
                    for ki in range(2):
                        nc.tensor.matmul(vps[:], p1aw[:, ki, mo * 128:(mo + 1) * 128],
                                         gmax_bf[ki][:], start=(ki == 0), stop=(ki == 1))
                    nc.vector.tensor_scalar(vb1[mo][:], vps[:], p1b[k][:, mo:mo + 1],
                                            None, AL.add)

                # ---------------- I: pred1
                p1w = wpool.tile([128, 8, 256], BF16, tag="p1")
                nc.sync.dma_start(out=p1w[:], in_=wk['p1_w'][:].rearrange("s k m -> k s m"))
                for h in range(NH):
                    for cch in range(NCH):
                        g0 = h * HI + cch * 4
                        for mo in range(2):
                            pp = psum.tile([128, 4 * 128], F32, tag="ps", space="PSUM")
                            for s in range(8):
                                rhs = X[s + 1][:, g0:g0 + 4, 16:144]
                                nc.tensor.matmul(pp[:], p1w[:, s, mo * 128:(mo + 1) * 128],
                                                 rhs, start=(s == 0), stop=(s == 7))
                            for j in range(4):
                                n = g0 + j
                                nc.scalar.activation(
                                    P1[mo][:, n, :], pp[:, j * 128:(j + 1) * 128],
                                    AF.Relu, bias=vb1[mo][:, n:n + 1], scale=1.0)

                # ---------------- J: pred2
                p2w = wpool.tile([128, 2, 64], BF16, tag="p2")
                nc.sync.dma_start(out=p2w[:], in_=wk['p2_w'][:].rearrange("s k m -> k s m"))
                for h in range(NH):
                    for cch in range(NCH):
                        g0 = h * HI + cch * 4
                        pp = psum.tile([64, 4 * 128], F32, tag="ps", space="PSUM")
                        for ki in range(2):
                            nc.tensor.matmul(pp[:], p2w[:, ki, :], P1[ki][:, g0:g0 + 4, :],
                                             start=(ki == 0), stop=(ki == 1))
                        nc.scalar.activation(P2[:64, g0:g0 + 4, :],
                                             pp[:].rearrange("p (g n) -> p g n", g=4),
                                             AF.Relu, bias=p2b[k][:, 0:1], scale=1.0)

                # ---------------- K: pred3 (per-instance lhsT) + py update
                p3w = wpool.tile([65, 2], BF16, tag="p3")
                nc.sync.dma_start(out=p3w[:], in_=wk['p3_w'][:])
                p3ps = psum.tile([128, n_inst * 2], F32, tag="ps", space="PSUM")
                for n in range(n_inst):
                    nc.tensor.matmul(p3ps[:, 2 * n:2 * n + 2], P2[:, n, :], p3w[:],
                                     start=True, stop=True, skip_group_check=True)
                nc.vector.tensor_scalar(py_img[:], py[:], 4.0, None, AL.mult)
                nc.vector.tensor_tensor(
                    out=py_img[:], in0=py_img[:],
                    in1=p3ps[:].rearrange("p (n c) -> p n c", c=2), op=AL.add)
                nc.sync.dma_start(
                    out=out_pys[k].rearrange("p (n c) -> p n c", c=2), in_=py_img[:])
                if k + 1 < n_iter:
                    nc.vector.tensor_scalar(py[:], py_img[:], 0.25, None, AL.mult)

    nc.compile()
    return nc


_PROGRAM_CACHE = {}


def kernel(cnn_feature, polys, ind, snake_params):
    cnn_feature = np.asarray(cnn_feature, np.float32)
    polys = np.asarray(polys, np.float32)
    ind = np.asarray(ind)
    n_total, n_pts = polys.shape[0], polys.shape[1]
    assert n_pts == PTS and cnn_feature.shape == (B, C, H, W)
    n_inst = n_total // N_CORES
    n_iter = len(snake_params)

    wmeta = prep_weights(snake_params)

    key = (n_inst, n_iter)
    if key not in _PROGRAM_CACHE:
        _PROGRAM_CACHE[key] = build_program(n_inst, n_iter, wmeta)
    nc = _PROGRAM_CACHE[key]

    # host data prep
    feat_t = np.ascontiguousarray(cnn_feature.transpose(0, 2, 3, 1)).reshape(-1, 64)
    feat_pad = np.zeros((FEAT_ROWS * 256 + 512,), bf16)
    feat_pad[:feat_t.size] = feat_t.reshape(-1).astype(bf16)

    wmap = {}
    for k in range(n_iter):
        m = wmeta[k]
        wmap[f"w{k}_head"] = np.zeros((9, 66, 128), bf16)
        wmap[f"w{k}_head"][:, :66, :] = m['taps'][0]
        for i in range(7):
            wmap[f"w{k}_res{i}"] = m['taps'][i + 1]
        wmap[f"w{k}_s"] = m['s']; wmap[f"w{k}_bs"] = m['bs']; wmap[f"w{k}_t"] = m['t']
        wmap[f"w{k}_fus"] = m['fus_w']; wmap[f"w{k}_fusb"] = m['fus_b']
        wmap[f"w{k}_p1"] = m['p1_w']; wmap[f"w{k}_p1a"] = m['p1a_w']
        wmap[f"w{k}_p1b"] = m['p1_b']
        wmap[f"w{k}_p2"] = m['p2_w']; wmap[f"w{k}_p2b"] = m['p2_b']
        wmap[f"w{k}_p3"] = m['p3_w']

    in_maps = []
    for c in range(N_CORES):
        sl = slice(c * n_inst, (c + 1) * n_inst)
        pp = polys[sl]  # [n_inst, 128, 2]
        polys_pt = np.ascontiguousarray(pp.transpose(1, 0, 2)).reshape(PTS, n_inst * 2)
        ind_rep = np.broadcast_to(ind[sl].astype(np.float32)[None, :],
                                  (PTS, n_inst)).copy()
        m = {"feat": feat_pad, "polys_pt": polys_pt, "ind_rep": ind_rep}
        m.update(wmap)
        in_maps.append(m)

    res = run_bass_kernel_spmd(nc, in_maps, core_ids=list(range(N_CORES)))
    outs = []
    for c in range(N_CORES):
        o = res.results[c]["out_pys"]  # [n_iter, 128, n_inst*2]
        outs.append(o.reshape(n_iter, PTS, n_inst, 2).transpose(0, 2, 1, 3))
    return np.concatenate(outs, axis=1).astype(np.float32)  # [n_iter, N, P, 2]
